# revision 1
# baseline (speedup 1.0000x reference)
"""Trainium2 Bass kernel: 3-layer GAT + BN + ELU + residual + global mean pool + linear.

Sharding: nodes (and their incident edges, grouped by destination) are
sharded across 8 NeuronCores. Weights replicated. Per layer:
  1. local h_ext = x_local @ [W | W@As | W@Ad]  (node-major rows)
  2. AllGather h_ext -> full [N, ROW] table in DRAM (bf16)
  3. per dst-block: dma_gather of h_ext[src] rows for this core's edges,
     attention weights w = exp(leaky(sS[src]+sD[dst])) via one-hot
     broadcast matmul; weighted scatter-matmul accumulates U and Z in
     PSUM; y = U/(Z+eps)
  4. BN stats (ones-matmul) -> AllReduce -> scale/shift -> ELU -> residual
Pool + final linear at the end (AllReduce of pooled sums).
"""
import sys
if '/opt/trn_rl_repo' not in sys.path:
    sys.path.insert(0, '/opt/trn_rl_repo')
import numpy as np
import ml_dtypes

import concourse.bass as bass
import concourse.bacc as bacc
import concourse.mybir as mybir
from concourse import tile
from concourse.bass_utils import run_bass_kernel_spmd

F32 = mybir.dt.float32
BF16 = mybir.dt.bfloat16
I16 = mybir.dt.int16
AL = mybir.AluOpType
ACTF = mybir.ActivationFunctionType
AX = mybir.AxisListType

N, E, FIN, H, C, G, NCLS = 10000, 160000, 512, 8, 64, 64, 64
P = 8
NL = N // P            # 1250 nodes per core
NT = 10                # node tiles per core (9x128 + 98)
LAST = NL - 9 * 128    # 98
CH = 6                 # gather-chunk size in 128-edge slots
ROW12 = 640            # bf16 gather row (640*2B = 1280B, %256==0); data in 0:528
ROW3 = 128             # bf16 gather row L3 (256B); data in 0:66
SS12 = 528             # h(512) | sS(8) | sD(8)
SS3 = 66               # h(64) | sS(1) | sD(1)
EPS_Z = 1e-16
EPS_BN = 1e-5
NP_BF16 = ml_dtypes.bfloat16


def _blockdiag(a):
    # a [H, C] -> [H*C, H] with column h holding a[h] in rows h*C:(h+1)*C
    hh, cc = a.shape
    out = np.zeros((hh * cc, hh), np.float64)
    for h in range(hh):
        out[h * cc:(h + 1) * cc, h] = a[h]
    return out


def _prep(inputs):
    x = np.asarray(inputs['x'], np.float32)
    ei = np.asarray(inputs['edge_index'], np.int64)
    batch = np.asarray(inputs['batch'], np.int64)

    src = np.concatenate([ei[0], np.arange(N, dtype=np.int64)])
    dst = np.concatenate([ei[1], np.arange(N, dtype=np.int64)])
    order = np.argsort(dst, kind='stable')
    src, dst = src[order], dst[order]

    core = dst // NL
    blk = (dst % NL) // 128
    dloc = (dst % NL) % 128

    per_cb = {}
    T = np.ones(NT, np.int64)
    for c in range(P):
        m = core == c
        sc, dc, bc = src[m], dloc[m], blk[m]
        for b in range(NT):
            mb = bc == b
            per_cb[(c, b)] = (sc[mb], dc[mb])
            T[b] = max(T[b], (int(mb.sum()) + 127) // 128)
    sbase = np.zeros(NT, np.int64)
    sbase[1:] = np.cumsum(T)[:-1]
    TT = int(T.sum())
    NE = TT * 128

    per_core = []
    for c in range(P):
        sidx = np.zeros(NE, np.int64)
        dl = np.full(NE, 255, np.int64)
        for b in range(NT):
            sc, dc = per_cb[(c, b)]
            off = int(sbase[b]) * 128
            sidx[off:off + len(sc)] = sc
            dl[off:off + len(dc)] = dc
        j = np.arange(NE)
        t, pp = j // 128, j % 128
        valid = dl < 128
        S = np.zeros((TT, 128, 128), NP_BF16)
        S[t[valid], pp[valid], dl[valid]] = 1
        ST = np.ascontiguousarray(S.transpose(0, 2, 1))
        # chunk-major layouts for contiguous per-partition DMA
        chunks = []
        for b in range(NT):
            s0 = int(sbase[b])
            for c0 in range(0, int(T[b]), CH):
                chunks.append((s0 + c0, min(CH, int(T[b]) - c0)))
        SC = np.zeros((len(chunks), 128, CH * 128), NP_BF16)
        STC = np.zeros((len(chunks), 128, CH * 128), NP_BF16)
        for ci, (gs0, nsl) in enumerate(chunks):
            SC[ci, :, 0:nsl * 128] = S[gs0:gs0 + nsl].transpose(1, 0, 2
                ).reshape(128, nsl * 128)
            STC[ci, :, 0:nsl * 128] = ST[gs0:gs0 + nsl].transpose(1, 0, 2
                ).reshape(128, nsl * 128)
        g16 = np.zeros((16, NE // 16), np.int16)
        g16[j % 16, j // 16] = sidx.astype(np.int16)
        gidx = np.tile(g16, (8, 1))

        xc = x[c * NL:(c + 1) * NL]                      # [1250, 512]
        x0T = np.zeros((FIN, NT * 128), np.float32)
        x0T[:, :NL] = xc.T
        x0T = x0T.astype(NP_BF16)

        cnt = np.bincount(batch, minlength=G).astype(np.float64)
        inv = 1.0 / np.maximum(cnt, 1.0)
        ind = (cnt > 0).astype(np.float32)
        pool = np.zeros((NT, 128, G), np.float32)
        nodes = np.arange(NL) + c * NL
        nn, ppp = np.arange(NL) // 128, np.arange(NL) % 128
        pool[nn, ppp, batch[nodes]] = inv[batch[nodes]]

        per_core.append(dict(S=SC, ST=STC, gidx=gidx, x0T=x0T,
                             pool=pool))

    f64 = lambda k: np.asarray(inputs[k], np.float64)
    W1, W2, W3 = f64('W1'), f64('W2'), f64('W3')
    Wcat1 = np.concatenate(
        [W1, W1 @ _blockdiag(f64('as1')), W1 @ _blockdiag(f64('ad1'))], axis=1)
    Wcat2 = np.concatenate(
        [W2, W2 @ _blockdiag(f64('as2')), W2 @ _blockdiag(f64('ad2'))], axis=1)
    Wcat3 = np.concatenate(
        [W3, (W3 @ f64('as3')[0])[:, None], (W3 @ f64('ad3')[0])[:, None]],
        axis=1)
    encW = f64('enc_W')
    RHS0 = np.concatenate([encW, encW @ Wcat1], axis=1)       # [512, 1040]
    eb1 = (f64('enc_b') @ Wcat1)[None, :]                      # [1, 528]

    shared = dict(
        rhs0=RHS0.astype(NP_BF16),
        w2=Wcat2.astype(NP_BF16),
        w3=Wcat3.astype(NP_BF16),
        encb=np.asarray(inputs['enc_b'], np.float32)[None, :],
        eb1=eb1.astype(np.float32),
        g1=np.asarray(inputs['g1'], np.float32)[None, :],
        be1=np.asarray(inputs['be1'], np.float32)[None, :],
        g2=np.asarray(inputs['g2'], np.float32)[None, :],
        be2=np.asarray(inputs['be2'], np.float32)[None, :],
        g3=np.asarray(inputs['g3'], np.float32)[None, :],
        be3=np.asarray(inputs['be3'], np.float32)[None, :],
        linW=np.asarray(inputs['lin_W'], np.float32),
        linb=np.asarray(inputs['lin_b'], np.float32)[:, None],
        ident=np.eye(128, dtype=np.float32),
        indmat=np.broadcast_to((np.bincount(np.asarray(inputs['batch'],
            np.int64), minlength=G) > 0).astype(np.float32)[None, :],
            (C, G)).copy(),
    )
    return T.tolist(), TT, len(chunks), per_core, shared


def _build(T_list, TT, NCH, repeat=1):
    nc = bacc.Bacc(None, target_bir_lowering=False, debug=False, num_devices=P,
                   num_swdge_queues=2)
    NE = TT * 128
    sbase = [0] * NT
    for b in range(1, NT):
        sbase[b] = sbase[b - 1] + T_list[b - 1]
    TMAXB = max(T_list)
    chunk_of = {}
    _ci = 0
    for _b in range(NT):
        for _c0 in range(0, T_list[_b], CH):
            chunk_of[(_b, _c0)] = _ci
            _ci += 1
    assert _ci == NCH

    # ---- external inputs ----
    S_d = nc.dram_tensor("S", [NCH, 128, CH * 128], BF16, kind="ExternalInput")
    ST_d = nc.dram_tensor("ST", [NCH, 128, CH * 128], BF16, kind="ExternalInput")
    gidx_d = nc.dram_tensor("gidx", [128, NE // 16], I16, kind="ExternalInput")
    x0T_d = nc.dram_tensor("x0T", [FIN, NT * 128], BF16, kind="ExternalInput")
    pool_d = nc.dram_tensor("pool", [NT, 128, G], F32, kind="ExternalInput")
    rhs0_d = nc.dram_tensor("rhs0", [FIN, 1040], BF16, kind="ExternalInput")
    w2_d = nc.dram_tensor("w2", [FIN, SS12], BF16, kind="ExternalInput")
    w3_d = nc.dram_tensor("w3", [FIN, SS3], BF16, kind="ExternalInput")
    encb_d = nc.dram_tensor("encb", [1, FIN], F32, kind="ExternalInput")
    eb1_d = nc.dram_tensor("eb1", [1, SS12], F32, kind="ExternalInput")
    bn_d = {}
    for ly, wd in ((1, FIN), (2, FIN), (3, C)):
        bn_d[ly] = (nc.dram_tensor(f"g{ly}", [1, wd], F32, kind="ExternalInput"),
                    nc.dram_tensor(f"be{ly}", [1, wd], F32, kind="ExternalInput"))
    linW_d = nc.dram_tensor("linW", [C, NCLS], F32, kind="ExternalInput")
    linb_d = nc.dram_tensor("linb", [NCLS, 1], F32, kind="ExternalInput")
    ident_d = nc.dram_tensor("ident", [128, 128], F32, kind="ExternalInput")
    indmat_d = nc.dram_tensor("indmat", [C, G], F32, kind="ExternalInput")
    out_d = nc.dram_tensor("out", [G, NCLS], F32, kind="ExternalOutput")

    # ---- internal DRAM ----
    cc_in = {1: nc.dram_tensor("cc_in1", [NL, ROW12], BF16),
             2: nc.dram_tensor("cc_in2", [NL, ROW12], BF16),
             3: nc.dram_tensor("cc_in3", [NL, ROW3], BF16)}
    cc_out = {1: nc.dram_tensor("cc_out1", [N, ROW12], BF16, addr_space="Shared"),
              2: nc.dram_tensor("cc_out2", [N, ROW12], BF16, addr_space="Shared"),
              3: nc.dram_tensor("cc_out3", [N, ROW3], BF16, addr_space="Shared")}
    st_in = {1: nc.dram_tensor("st_in1", [1, 2 * FIN], F32),
             2: nc.dram_tensor("st_in2", [1, 2 * FIN], F32),
             3: nc.dram_tensor("st_in3", [1, 2 * C], F32)}
    st_out = {1: nc.dram_tensor("st_out1", [P, 2 * FIN], F32, addr_space="Shared"),
              2: nc.dram_tensor("st_out2", [P, 2 * FIN], F32, addr_space="Shared"),
              3: nc.dram_tensor("st_out3", [P, 2 * C], F32, addr_space="Shared")}
    ar3_in = nc.dram_tensor("ar3_in", [C + 2, G], F32)
    ar3_out = nc.dram_tensor("ar3_out", [(C + 2) * P, G], F32, addr_space="Shared")
    RG = [list(range(P))]

    with tile.TileContext(nc) as tc:
        with tc.tile_pool(name="cn", bufs=1) as cn, \
             tc.tile_pool(name="xb", bufs=1) as xb, \
             tc.tile_pool(name="gp", bufs=4) as gp, \
             tc.tile_pool(name="sp", bufs=4) as sp, \
             tc.tile_pool(name="wp", bufs=2) as wp, \
             tc.tile_pool(name="sm", bufs=2) as sm, \
             tc.tile_pool(name="psA", bufs=2, space="PSUM") as psA, \
             tc.tile_pool(name="psB", bufs=1, space="PSUM") as psB, \
             tc.tile_pool(name="psU", bufs=3, space="PSUM") as psU:

            # ---- load constants ----
            def cload(name, shape, dtype, dram, rearr=None, eng=None, **kw):
                t = cn.tile(shape, dtype, tag=name)
                src = dram[:] if rearr is None else dram[:].rearrange(rearr, **kw)
                (eng or nc.gpsimd).dma_start(t[:], src)
                return t

            idx_sb = cload("idx", [128, NE // 16], I16, gidx_d, eng=nc.sync)
            pool_sb = cload("pool", [128, NT, G], F32, pool_d, "n p g -> p n g")
            ident_sb = cload("ident", [128, 128], F32, ident_d)
            encb_sb = cload("encb", [1, FIN], F32, encb_d, eng=nc.sync)
            eb1_sb = cload("eb1", [1, SS12], F32, eb1_d, eng=nc.sync)
            w3_sb = cload("w3", [128, 4, SS3], BF16, w3_d, "(k p) x -> p k x", p=128)
            linW_sb = cload("linW", [C, NCLS], F32, linW_d)
            indmat_sb = cload("indmat", [C, G], F32, indmat_d)
            linb_sb = cload("linb", [NCLS, 1], F32, linb_d)
            bn_sb = {ly: (cload(f"g{ly}", [1, wd], F32, bn_d[ly][0]),
                          cload(f"be{ly}", [1, wd], F32, bn_d[ly][1]))
                     for ly, wd in ((1, FIN), (2, FIN), (3, C))}

            ones_c = cn.tile([128, 1], BF16, tag="ones_c")
            nc.vector.memset(ones_c[:], 1.0)
            ones_cf = cn.tile([128, 1], F32, tag="ones_cf")
            nc.vector.memset(ones_cf[:], 1.0)
            ones_r = cn.tile([1, 128], F32, tag="ones_r")
            nc.vector.memset(ones_r[:], 1.0)
            epsbn_t = cn.tile([1, 1], F32, tag="epsbn")
            nc.vector.memset(epsbn_t[:], EPS_BN)
            zeros_t = cn.tile([128, 1], F32, tag="zeros_t")
            nc.vector.memset(zeros_t[:], 0.0)

            # big rotating node-feature buffers (f32)
            bufs = [xb.tile([128, NT, FIN], F32, tag=f"big{i}", name=f"big{i}") for i in range(3)]

            def nvalid(n):
                return 128 if n < NT - 1 else LAST

            # ---------- h_ext matmul phase ----------
            def h_phase(ly, lhsT_sb, wcat_sb, wofs, ss, rowv, bias_sb, xe_buf,
                        sdloc):
                for n in range(NT):
                    ht = sm.tile([128, SS12], BF16, tag="hrow")
                    p5 = psA.tile([128, FIN], F32, tag="mm5")
                    pS = psB.tile([128, SS3], F32, tag="Z", name="pS")
                    nh = ss - FIN if ly < 3 else 2
                    fh = FIN if ly < 3 else C
                    for k in range(4):
                        lt = lhsT_sb[:, k, 128 * n:128 * (n + 1)]
                        if ly < 3:
                            nc.tensor.matmul(p5[:], lt,
                                             wcat_sb[:, k, wofs:wofs + FIN],
                                             start=(k == 0), stop=(k == 3 and ly == 2))
                            nc.tensor.matmul(pS[:, 0:16], lt,
                                             wcat_sb[:, k, wofs + FIN:wofs + ss],
                                             start=(k == 0), stop=(k == 3 and ly == 2))
                        else:
                            nc.tensor.matmul(pS[:, 0:SS3], lt,
                                             wcat_sb[:, k, 0:SS3],
                                             start=(k == 0), stop=(k == 3))
                    if ly == 1:
                        nc.tensor.matmul(p5[:], ones_r[:],
                                         bias_sb[:, 0:FIN],
                                         start=False, stop=True,
                                         skip_group_check=True)
                        nc.tensor.matmul(pS[:, 0:16], ones_r[:],
                                         bias_sb[:, FIN:ss],
                                         start=False, stop=True,
                                         skip_group_check=True)
                    if ly < 3:
                        nc.scalar.copy(ht[:, 0:FIN], p5[:])
                        nc.scalar.copy(ht[:, FIN:ss], pS[:, 0:16])
                        nc.scalar.copy(sdloc[:, n, :], pS[:, 8:16])
                    else:
                        nc.scalar.copy(ht[:, 0:SS3], pS[:, 0:SS3])
                        nc.scalar.copy(sdloc[:, n, :], pS[:, 65:66])
                    v = nvalid(n)
                    nc.sync.dma_start(
                        cc_in[ly][128 * n:128 * n + v, 0:ss], ht[0:v, 0:ss])
                if ly == 1:
                    # xe (residual base) deprioritized: not needed until the
                    # BN chain, so it fills PE gaps during the edge phase
                    for n in range(NT):
                        pxe = psA.tile([128, FIN], F32, tag="mm5", name="pxe")
                        for k in range(4):
                            nc.tensor.matmul(
                                pxe[:], lhsT_sb[:, k, 128 * n:128 * (n + 1)],
                                wcat_sb[:, k, 0:FIN], start=(k == 0),
                                stop=False)
                        nc.tensor.matmul(pxe[:], ones_r[:], encb_sb[:],
                                         start=False, stop=True,
                                         skip_group_check=True)
                        nc.scalar.copy(xe_buf[:, n, :], pxe[:])

            # ---------- edge aggregation phase ----------
            def edge_phase(ly, rowv, ss, nh, fh, sdloc, ybuf, pstA, pstB):
                cph = fh // nh
                for b in range(NT):
                    T = T_list[b]
                    s0 = sbase[b]
                    w_t = wp.tile([128, TMAXB, 8], BF16, tag="w_t")
                    first = True
                    pU = psU.tile([128, FIN], F32, tag="U")
                    pZ = psB.tile([128, 8], F32, tag="Z")
                    for c0 in range(0, T, CH):
                        nsl = min(CH, T - c0)
                        sg = s0 + c0
                        g = gp.tile([128, CH, ROW12 if ly < 3 else ROW3],
                                    BF16, tag="g")
                        nc.gpsimd.dma_gather(
                            g[:, 0:nsl, 0:rowv], cc_out[ly][:],
                            idx_sb[:, 8 * sg:8 * (sg + nsl)],
                            num_idxs=nsl * 128, num_idxs_reg=nsl * 128,
                            elem_size=rowv, queue_num=(c0 // CH) % 2)
                        Ssb = sp.tile([128, CH, 128], BF16, tag="S")
                        STsb = sp.tile([128, CH, 128], BF16, tag="ST")
                        ci = chunk_of[(b, c0)]
                        nc.sync.dma_start(Ssb[:, 0:nsl, :],
                                          S_d[ci, :, 0:nsl * 128])
                        nc.sync.dma_start(STsb[:, 0:nsl, :],
                                          ST_d[ci, :, 0:nsl * 128])
                        psd = psB.tile([128, CH * 8], F32, tag="sd", bufs=2)
                        for t in range(nsl):
                            nc.tensor.matmul(
                                psd[:, t * nh:(t + 1) * nh], STsb[:, t, :],
                                sdloc[:, b, :], start=True, stop=True)
                        lg = wp.tile([128, CH * 8], F32, tag="lg")
                        nc.vector.tensor_tensor(
                            out=lg[:, 0:nsl * nh],
                            in0=g[:, 0:nsl, fh:fh + nh],
                            in1=psd[:, 0:nsl * nh], op=AL.add)
                        nc.vector.scalar_tensor_tensor(
                            out=lg[:, 0:nsl * nh], in0=lg[:, 0:nsl * nh],
                            scalar=0.2, in1=lg[:, 0:nsl * nh],
                            op0=AL.mult, op1=AL.max)
                        nc.scalar.activation(
                            w_t[:, c0:c0 + nsl, 0:nh], lg[:, 0:nsl * nh],
                            ACTF.Exp)
                        if nh == 8:
                            nc.vector.tensor_tensor(
                                out=g[:, 0:nsl, 0:6 * cph],
                                in0=g[:, 0:nsl, 0:6 * cph],
                                in1=w_t[:, c0:c0 + nsl, 0:6].unsqueeze(3)
                                    .broadcast_to([128, nsl, 6, cph]),
                                op=AL.mult)
                            nc.gpsimd.tensor_tensor(
                                out=g[:, 0:nsl, 6 * cph:fh],
                                in0=g[:, 0:nsl, 6 * cph:fh],
                                in1=w_t[:, c0:c0 + nsl, 6:8].unsqueeze(3)
                                    .broadcast_to([128, nsl, 2, cph]),
                                op=AL.mult)
                        else:
                            nc.vector.tensor_tensor(
                                out=g[:, 0:nsl, 0:fh], in0=g[:, 0:nsl, 0:fh],
                                in1=w_t[:, c0:c0 + nsl, 0:nh].unsqueeze(3)
                                    .broadcast_to([128, nsl, nh, cph]),
                                op=AL.mult)
                        for t in range(nsl):
                            nc.tensor.matmul(
                                pU[:, 0:fh], Ssb[:, t, :], g[:, t, 0:fh],
                                start=first, stop=(c0 + t == T - 1),
                                skip_group_check=True)
                            nc.tensor.matmul(
                                pZ[:, 0:nh], Ssb[:, t, :],
                                w_t[:, c0 + t, 0:nh],
                                start=first, stop=(c0 + t == T - 1),
                                skip_group_check=True)
                            first = False
                    rz = sm.tile([128, 8], F32, tag="rz")
                    nc.vector.tensor_scalar_add(rz[:, 0:nh], pZ[:, 0:nh], EPS_Z)
                    nc.vector.reciprocal(rz[:, 0:nh], rz[:, 0:nh])
                    nc.vector.tensor_tensor(
                        out=ybuf[:, b, 0:fh], in0=pU[:, 0:fh],
                        in1=rz[:, 0:nh].unsqueeze(2)
                            .broadcast_to([128, nh, cph]),
                        op=AL.mult)
                    y2 = sm.tile([128, FIN], F32, tag="y2")
                    nc.scalar.activation(y2[:, 0:fh], ybuf[:, b, 0:fh],
                                         ACTF.Square)
                    nc.tensor.matmul(pstA[:, 0:fh], ones_cf[:],
                                     ybuf[:, b, 0:fh], start=(b == 0),
                                     stop=(b == NT - 1),
                                     skip_group_check=True)
                    nc.tensor.matmul(pstB[:, 0:fh], ones_cf[:], y2[:, 0:fh],
                                     start=(b == 0), stop=(b == NT - 1),
                                     skip_group_check=True)

            # ---------- BN + (ELU + residual) ----------
            def bn_chain(ly, fh, ybuf, xprev, xnext, pst1, pst2):
                g_sb, be_sb = bn_sb[ly]
                stat = sm.tile([1, 2 * FIN], F32, tag="stat")
                nc.scalar.copy(stat[:, 0:fh], pst1[:, 0:fh])
                nc.scalar.copy(stat[:, fh:2 * fh], pst2[:, 0:fh])
                nc.sync.dma_start(st_in[ly][:], stat[:, 0:2 * fh])
                nc.gpsimd.collective_compute(
                    "AllGather", AL.bypass, replica_groups=RG,
                    ins=[st_in[ly][:]], outs=[st_out[ly][:]])
                st8 = sm.tile([P, 2 * FIN], F32, tag="st8", bufs=1)
                nc.sync.dma_start(st8[:, 0:2 * fh], st_out[ly][:, 0:2 * fh])
                pm1 = psA.tile([1, FIN], F32, tag="mm5", name="pm1")
                pm2 = psA.tile([1, FIN], F32, tag="mm5", name="pm2")
                nc.tensor.matmul(pm1[:, 0:fh], ones_cf[0:P, :],
                                 st8[:, 0:fh], start=True, stop=True)
                nc.tensor.matmul(pm2[:, 0:fh], ones_cf[0:P, :],
                                 st8[:, fh:2 * fh], start=True, stop=True)
                st2 = sm.tile([1, 2 * FIN], F32, tag="stat2")
                nc.scalar.copy(st2[:, 0:fh], pm1[:, 0:fh])
                nc.scalar.copy(st2[:, fh:2 * fh], pm2[:, 0:fh])
                mu = st2[:, 0:fh]
                ex2 = st2[:, fh:2 * fh]
                nc.vector.tensor_scalar_mul(mu, mu, 1.0 / N)
                nc.vector.tensor_scalar_mul(ex2, ex2, 1.0 / N)
                var = sm.tile([1, FIN], F32, tag="var")
                nc.vector.tensor_tensor(out=var[:, 0:fh], in0=mu, in1=mu,
                                        op=AL.mult)
                nc.vector.tensor_tensor(out=var[:, 0:fh], in0=ex2,
                                        in1=var[:, 0:fh], op=AL.subtract)
                sd = sm.tile([1, FIN], F32, tag="sdv")
                nc.scalar.activation(sd[:, 0:fh], var[:, 0:fh], ACTF.Sqrt,
                                     bias=epsbn_t[:])
                nc.vector.reciprocal(sd[:, 0:fh], sd[:, 0:fh])
                scf = sm.tile([1, FIN], F32, tag="scf")
                nc.vector.tensor_tensor(out=scf[:, 0:fh], in0=g_sb[:],
                                        in1=sd[:, 0:fh], op=AL.mult)
                shf = sm.tile([1, FIN], F32, tag="shf")
                nc.vector.tensor_tensor(out=shf[:, 0:fh], in0=scf[:, 0:fh],
                                        in1=mu, op=AL.mult)
                nc.vector.tensor_tensor(out=shf[:, 0:fh], in0=be_sb[:],
                                        in1=shf[:, 0:fh], op=AL.subtract)
                scT = sm.tile([128, FIN], F32, tag="scT")
                shT = sm.tile([128, FIN], F32, tag="shT")
                nc.gpsimd.partition_broadcast(scT[:, 0:fh], scf[:, 0:fh])
                nc.gpsimd.partition_broadcast(shT[:, 0:fh], shf[:, 0:fh])
                for n in range(NT):
                    eng = nc.vector
                    eng1 = nc.gpsimd
                    v = sm.tile([128, FIN], F32, tag="cht", name="v")
                    eng1.tensor_tensor(out=v[:, 0:fh],
                                       in0=ybuf[:, n, 0:fh],
                                       in1=scT[:, 0:fh], op=AL.mult)
                    eng1.tensor_tensor(out=v[:, 0:fh], in0=v[:, 0:fh],
                                       in1=shT[:, 0:fh], op=AL.add)
                    if ly == 3:
                        eng.tensor_copy(xnext[:, n, 0:fh], v[:, 0:fh])
                        continue
                    m = sm.tile([128, FIN], F32, tag="che", name="m")
                    eng.tensor_scalar_min(m[:, 0:fh], v[:, 0:fh], 0.0)
                    nc.scalar.activation(m[:, 0:fh], m[:, 0:fh], ACTF.Exp)
                    eng.scalar_tensor_tensor(
                        out=v[:, 0:fh], in0=v[:, 0:fh], scalar=0.0,
                        in1=m[:, 0:fh], op0=AL.max, op1=AL.add)
                    eng.scalar_tensor_tensor(
                        out=xnext[:, n, 0:fh], in0=v[:, 0:fh], scalar=-1.0,
                        in1=xprev[:, n, 0:fh], op0=AL.add, op1=AL.add)

            # ---------- transpose a -> aT (bf16) ----------
            def transpose_phase(abuf, aT):
                for n in range(NT):
                    for k in range(4):
                        tr = psB.tile([128, 128], F32, tag="sd", bufs=2, name="tr")
                        nc.tensor.transpose(
                            tr[:], abuf[:, n, 128 * k:128 * (k + 1)],
                            ident_sb[:])
                        nc.scalar.copy(
                            aT[:, k, 128 * n:128 * (n + 1)], tr[:])

            # =========== emit program ===========
            for _rep in range(repeat):
              xe, ybuf1, a1 = bufs[0], bufs[1], bufs[2]
              x0T_sb = xb.tile([128, 4, NT * 128], BF16, tag="lhsT",
                               name="x0T_sb")
              nc.sync.dma_start(x0T_sb[:],
                                x0T_d[:].rearrange("(k p) x -> p k x", p=128))
              sdloc = xb.tile([128, NT, 8], BF16, tag="sdloc", name="sdloc")
              wcat0 = cn.tile([128, 4, 1040], BF16, tag="wcat")
              nc.sync.dma_start(wcat0[:], rhs0_d[:].rearrange("(k p) x -> p k x", p=128))

              # encoder + L1 h
              h_phase(1, x0T_sb, wcat0, FIN, SS12, ROW12, eb1_sb, xe, sdloc)
              nc.gpsimd.collective_compute(
                  "AllGather", AL.bypass, replica_groups=RG,
                  ins=[cc_in[1][:]], outs=[cc_out[1][:]])
              pstA1 = psA.tile([1, FIN], F32, tag="mm5", name="pstA1")
              pstB1 = psA.tile([1, FIN], F32, tag="mm5", name="pstB1")
              edge_phase(1, ROW12, SS12, H, FIN, sdloc, ybuf1, pstA1, pstB1)
              bn_chain(1, FIN, ybuf1, xe, a1, pstA1, pstB1)

              # L2: a1 -> aT, h, edges (reuse xe buf as ybuf2, ybuf1 as a2)
              aT = xb.tile([128, 4, NT * 128], BF16, tag="lhsT")
              transpose_phase(a1, aT)
              wcat2 = cn.tile([128, 4, SS12], BF16, tag="wcat")
              nc.sync.dma_start(wcat2[:], w2_d[:].rearrange("(k p) x -> p k x", p=128))
              sdloc2 = xb.tile([128, NT, 8], BF16, tag="sdloc")
              h_phase(2, aT, wcat2, 0, SS12, ROW12, None, None, sdloc2)
              nc.gpsimd.collective_compute(
                  "AllGather", AL.bypass, replica_groups=RG,
                  ins=[cc_in[2][:]], outs=[cc_out[2][:]])
              ybuf2, a2 = xe, ybuf1
              pstA2 = psA.tile([1, FIN], F32, tag="mm5", name="pstA2")
              pstB2 = psA.tile([1, FIN], F32, tag="mm5", name="pstB2")
              edge_phase(2, ROW12, SS12, H, FIN, sdloc2, ybuf2, pstA2, pstB2)
              bn_chain(2, FIN, ybuf2, a1, a2, pstA2, pstB2)

              # L3
              aT2 = xb.tile([128, 4, NT * 128], BF16, tag="lhsT")
              transpose_phase(a2, aT2)
              sdloc3 = xb.tile([128, NT, 1], BF16, tag="sdloc")
              h_phase(3, aT2, w3_sb, 0, SS3, ROW3, None, None, sdloc3)
              nc.gpsimd.collective_compute(
                  "AllGather", AL.bypass, replica_groups=RG,
                  ins=[cc_in[3][:]], outs=[cc_out[3][:]])
              y3 = xb.tile([128, NT, C], F32, tag="y3")
              pstA3 = psA.tile([1, FIN], F32, tag="mm5", name="pstA3")
              pstB3 = psA.tile([1, FIN], F32, tag="mm5", name="pstB3")
              edge_phase(3, ROW3, SS3, 1, C, sdloc3, y3, pstA3, pstB3)

              # L3 stats + pooled sums, one AllReduce for both
              stat3 = sm.tile([1, 2 * FIN], F32, tag="stat", name="stat3")
              nc.scalar.copy(stat3[:, 0:C], pstA3[:, 0:C])
              nc.scalar.copy(stat3[:, C:2 * C], pstB3[:, 0:C])
              # pooling on pre-BN y3: ygT[f, g] = sum_n y3[n, f] pool[n, g]
              pxg = psB.tile([C, G], F32, tag="sd", bufs=2, name="pxg")
              for n in range(NT):
                  nc.tensor.matmul(pxg[:], y3[:, n, :], pool_sb[:, n, :],
                                   start=(n == 0), stop=(n == NT - 1))
              xg = sm.tile([C, G], F32, tag="xg")
              nc.scalar.copy(xg[:], pxg[:])
              nc.sync.dma_start(ar3_in[0:C, :], xg[:])
              nc.sync.dma_start(ar3_in[C:C + 1, :], stat3[:, 0:C])
              nc.sync.dma_start(ar3_in[C + 1:C + 2, :], stat3[:, C:2 * C])
              nc.gpsimd.collective_compute(
                  "AllGather", AL.bypass, replica_groups=RG,
                  ins=[ar3_in[:]], outs=[ar3_out[:]])
              pooled8 = sm.tile([C, P, G], F32, tag="pooled8", bufs=1)
              nc.sync.dma_start(
                  pooled8[:, :, :],
                  ar3_out[:].rearrange("(r i) g -> i r g", r=P)[0:C])
              yg2 = sm.tile([C, G], F32, tag="xg2")
              nc.vector.tensor_reduce(
                  out=yg2[:, :],
                  in_=pooled8[:, :, :].rearrange("i r g -> i g r"),
                  axis=AX.X, op=AL.add)
              st8b = sm.tile([P, 2 * C], F32, tag="st8", bufs=1, name="st8b")
              nc.sync.dma_start(
                  st8b[:, :],
                  ar3_out[:].rearrange("(r i) g -> r (i g)", r=P)
                  [:, C * G:C * G + 2 * C])
              pm3 = psA.tile([1, FIN], F32, tag="mm5", name="pm3")
              nc.tensor.matmul(pm3[:, 0:2 * C], ones_cf[0:P, :],
                               st8b[:, :], start=True, stop=True)
              st3 = sm.tile([1, 2 * FIN], F32, tag="stat2", name="st3")
              nc.scalar.copy(st3[:, 0:2 * C], pm3[:, 0:2 * C])
              mu3 = st3[:, 0:C]
              ex23 = st3[:, C:2 * C]
              nc.vector.tensor_scalar_mul(mu3, mu3, 1.0 / N)
              nc.vector.tensor_scalar_mul(ex23, ex23, 1.0 / N)
              var3 = sm.tile([1, FIN], F32, tag="var", name="var3")
              nc.vector.tensor_tensor(out=var3[:, 0:C], in0=mu3, in1=mu3,
                                      op=AL.mult)
              nc.vector.tensor_tensor(out=var3[:, 0:C], in0=ex23,
                                      in1=var3[:, 0:C], op=AL.subtract)
              sd3 = sm.tile([1, FIN], F32, tag="sdv", name="sd3")
              nc.scalar.activation(sd3[:, 0:C], var3[:, 0:C], ACTF.Sqrt,
                                   bias=epsbn_t[:])
              nc.vector.reciprocal(sd3[:, 0:C], sd3[:, 0:C])
              g3_sb, be3_sb = bn_sb[3]
              scf3 = sm.tile([1, FIN], F32, tag="scf", name="scf3")
              nc.vector.tensor_tensor(out=scf3[:, 0:C], in0=g3_sb[:],
                                      in1=sd3[:, 0:C], op=AL.mult)
              shf3 = sm.tile([1, FIN], F32, tag="shf", name="shf3")
              nc.vector.tensor_tensor(out=shf3[:, 0:C], in0=scf3[:, 0:C],
                                      in1=mu3, op=AL.mult)
              nc.vector.tensor_tensor(out=shf3[:, 0:C], in0=be3_sb[:],
                                      in1=shf3[:, 0:C], op=AL.subtract)
              # transpose scf3/shf3 rows into per-partition columns [C, 1]
              psc = psB.tile([C, 1], F32, tag="Z", name="psc")
              nc.tensor.transpose(psc[:], scf3[:, 0:C], ident_sb[0:1, 0:1])
              scol = sm.tile([C, 1], F32, tag="scol", name="scol")
              nc.scalar.copy(scol[:], psc[:])
              psh = psB.tile([C, 1], F32, tag="Z", name="psh")
              nc.tensor.transpose(psh[:], shf3[:, 0:C], ident_sb[0:1, 0:1])
              shcol = sm.tile([C, 1], F32, tag="shcol", name="shcol")
              nc.scalar.copy(shcol[:], psh[:])
              # xgbn = yg2 * scol + shcol * indmat
              sh_t = sm.tile([C, G], F32, tag="shterm", name="sh_t")
              nc.vector.tensor_scalar_mul(sh_t[:], indmat_sb[:], shcol[:])
              xgbn = sm.tile([C, G], F32, tag="xgbn", name="xgbn")
              nc.vector.scalar_tensor_tensor(
                  out=xgbn[:], in0=yg2[:], scalar=scol[:], in1=sh_t[:],
                  op0=AL.mult, op1=AL.add)
              # outT[nc, g] = linW.T @ xgbn  (contract over f)
              pot = psB.tile([NCLS, G], F32, tag="sd", bufs=2, name="pot")
              nc.tensor.matmul(pot[:], linW_sb[:], xgbn[:], start=True,
                               stop=True)
              outT = sm.tile([NCLS, G], F32, tag="outT")
              nc.scalar.activation(outT[:], pot[:], ACTF.Identity,
                                   bias=linb_sb[:])
              pfin = psB.tile([G, NCLS], F32, tag="sd", bufs=2, name="pfin")
              nc.tensor.transpose(pfin[:], outT[:], ident_sb[0:NCLS, 0:NCLS])
              fin = sm.tile([G, NCLS], F32, tag="fin_sb")
              nc.vector.tensor_copy(fin[:], pfin[:])
              nc.sync.dma_start(out_d[:], fin[:])

        sched_state, snap = tc.schedule_and_allocate()
        nc._sched_state = sched_state
        nc._pred_ns = snap.time

    nc.finalize()
    return nc


_CACHE = {}


def _get_nc(T_key, TT, NCH, repeat=1):
    key = (T_key, repeat)
    if key not in _CACHE:
        _CACHE[key] = _build(list(T_key), TT, NCH, repeat)
    return _CACHE[key]


def make_in_maps(per_core, shared):
    return [dict(S=pc['S'], ST=pc['ST'], gidx=pc['gidx'],
                 x0T=pc['x0T'], pool=pc['pool'], **shared)
            for pc in per_core]


def kernel(**inputs):
    T_list, TT, NCH, per_core, shared = _prep(inputs)
    nc = _get_nc(tuple(T_list), TT, NCH)
    in_maps = make_in_maps(per_core, shared)
    res = run_bass_kernel_spmd(nc, in_maps, core_ids=list(range(P)))
    return np.asarray(res.results[0]['out'], np.float32)



# revision 27
# speedup vs baseline: 1.1569x; 1.1569x over previous
"""Trainium2 Bass kernel: 3-layer GAT + BN + ELU + residual + global mean pool + linear.

Sharding: nodes (and their incident edges, grouped by destination) are
sharded across 8 NeuronCores. Weights replicated. Per layer:
  1. local h_ext = x_local @ [W | W@As | W@Ad]  (node-major rows)
  2. AllGather h_ext -> full [N, ROW] table in DRAM (bf16)
  3. per dst-block: dma_gather of h_ext[src] rows for this core's edges,
     attention weights w = exp(leaky(sS[src]+sD[dst])) via one-hot
     broadcast matmul; weighted scatter-matmul accumulates U and Z in
     PSUM; y = U/(Z+eps)
  4. BN stats (ones-matmul) -> AllGather -> scale/shift -> ELU -> residual
Pool + final linear at the end (AllGather of pooled sums).

Perf notes (cost-model driven):
  - 512-wide features stored interleaved [c, h] (c-major) so the per-edge
    attention multiply has a packed last dim -> DVE 2x mode. The
    interleave is a pure host-side permutation of weight rows/cols.
  - One-hot S (edge->dst) and ST matrices are bf16-resident in SBUF for
    all 3 layers (loaded once).
  - One dma_gather per dst block (18 slots) to amortize the SWDGE fixed
    overhead on the Pool engine.
  - BN statistics matmuls run on bf16 copies (4x cheaper on PE than f32).
  - alpha-multiply alternates DVE/Pool by block to balance engine load.
"""
import sys
if '/opt/trn_rl_repo' not in sys.path:
    sys.path.insert(0, '/opt/trn_rl_repo')
import numpy as np
import ml_dtypes

import concourse.bass as bass
import concourse.bacc as bacc
import concourse.mybir as mybir
from concourse import tile
from concourse.bass_utils import run_bass_kernel_spmd

F32 = mybir.dt.float32
FP8 = mybir.dt.float8e4
BF16 = mybir.dt.bfloat16
I16 = mybir.dt.int16
AL = mybir.AluOpType
ACTF = mybir.ActivationFunctionType
AX = mybir.AxisListType

N, E, FIN, H, C, G, NCLS = 10000, 160000, 512, 8, 64, 64, 64
P = 8
NL = N // P            # 1250 nodes per core
NT = 10                # node tiles per core (9x128 + 98)
LAST = NL - 9 * 128    # 98
ROW12 = 640            # bf16 gather row (640*2B = 1280B, %256==0); data in 0:528
ROW3 = 128             # bf16 gather row L3 (256B); data in 0:66
SS12 = 528             # h(512 ilv) | sS(8) | sD(8)
SS3 = 66               # h(64) | sS(1) | sD(1)
EPS_Z = 1e-16
EPS_BN = 1e-5
NP_BF16 = ml_dtypes.bfloat16
NP_FP8 = ml_dtypes.float8_e4m3

# interleave permutation: ilv position c*8+h  <- std position h*64+c
PERM = np.arange(FIN).reshape(H, C).T.reshape(-1)   # PERM[c*8+h] = h*64+c


def _blockdiag(a):
    # a [H, C] -> [H*C, H] with column h holding a[h] in rows h*C:(h+1)*C
    hh, cc = a.shape
    out = np.zeros((hh * cc, hh), np.float64)
    for h in range(hh):
        out[h * cc:(h + 1) * cc, h] = a[h]
    return out


def _prep(inputs):
    x = np.asarray(inputs['x'], np.float32)
    ei = np.asarray(inputs['edge_index'], np.int64)
    batch = np.asarray(inputs['batch'], np.int64)

    src = np.concatenate([ei[0], np.arange(N, dtype=np.int64)])
    dst = np.concatenate([ei[1], np.arange(N, dtype=np.int64)])
    order = np.argsort(dst, kind='stable')
    src, dst = src[order], dst[order]

    core = dst // NL
    blk = (dst % NL) // 128
    dloc = (dst % NL) % 128

    per_cb = {}
    T = np.ones(NT, np.int64)
    for c in range(P):
        m = core == c
        sc, dc, bc = src[m], dloc[m], blk[m]
        for b in range(NT):
            mb = bc == b
            per_cb[(c, b)] = (sc[mb], dc[mb])
            T[b] = max(T[b], (int(mb.sum()) + 127) // 128)
    sbase = np.zeros(NT, np.int64)
    sbase[1:] = np.cumsum(T)[:-1]
    TT = int(T.sum())
    NE = TT * 128

    per_core = []
    for c in range(P):
        sidx = np.zeros(NE, np.int64)
        dl = np.full(NE, 255, np.int64)
        for b in range(NT):
            sc, dc = per_cb[(c, b)]
            off = int(sbase[b]) * 128
            sidx[off:off + len(sc)] = sc
            dl[off:off + len(dc)] = dc
        j = np.arange(NE)
        t, pp = j // 128, j % 128
        valid = dl < 128
        S = np.zeros((TT, 128, 128), NP_FP8)
        S[t[valid], pp[valid], dl[valid]] = 1
        # resident layouts: [128, TT*128]
        S_flat = np.ascontiguousarray(S.transpose(1, 0, 2).reshape(128, TT * 128))
        ST_flat = np.ascontiguousarray(S.transpose(2, 0, 1).reshape(128, TT * 128))
        g16 = np.zeros((16, NE // 16), np.int16)
        g16[j % 16, j // 16] = sidx.astype(np.int16)
        gidx = np.tile(g16, (8, 1))

        xc = x[c * NL:(c + 1) * NL]                      # [1250, 512]
        x0T = np.zeros((FIN, NT * 128), np.float32)
        x0T[:, :NL] = xc.T
        x0T = x0T.astype(NP_BF16)

        cnt = np.bincount(batch, minlength=G).astype(np.float64)
        inv = 1.0 / np.maximum(cnt, 1.0)
        pool = np.zeros((NT, 128, G), np.float32)
        nodes = np.arange(NL) + c * NL
        nn, ppp = np.arange(NL) // 128, np.arange(NL) % 128
        pool[nn, ppp, batch[nodes]] = inv[batch[nodes]]

        per_core.append(dict(S=S_flat, ST=ST_flat, gidx=gidx, x0T=x0T,
                             pool=pool))

    f64 = lambda k: np.asarray(inputs[k], np.float64)
    W1, W2, W3 = f64('W1'), f64('W2'), f64('W3')
    # std-basis cat weights, then permute for the interleaved layout:
    #  - 512-wide activation streams (enc out, a1, a2, y1, y2) live in ilv
    #  - Wcat1 consumes std(enc raw in)=x@encW... enc out is ilv so W1 rows perm
    Wcat1 = np.concatenate(
        [W1, W1 @ _blockdiag(f64('as1')), W1 @ _blockdiag(f64('ad1'))], axis=1)
    Wcat2 = np.concatenate(
        [W2, W2 @ _blockdiag(f64('as2')), W2 @ _blockdiag(f64('ad2'))], axis=1)
    Wcat3 = np.concatenate(
        [W3, (W3 @ f64('as3')[0])[:, None], (W3 @ f64('ad3')[0])[:, None]],
        axis=1)
    encW = f64('enc_W')
    # encoder part of RHS0: output cols in ilv
    enc_ilv = encW[:, PERM]
    # h1 part: encW(std out) @ Wcat1(std in); first 512 output cols -> ilv
    part2 = encW @ Wcat1
    part2 = np.concatenate([part2[:, PERM], part2[:, FIN:]], axis=1)
    RHS0 = np.concatenate([enc_ilv, part2], axis=1)          # [512, 1040]
    eb1 = (f64('enc_b') @ Wcat1)
    eb1 = np.concatenate([eb1[PERM], eb1[FIN:]])[None, :]     # [1, 528]
    encb_ilv = np.asarray(inputs['enc_b'], np.float64)[PERM]
    # Wcat2: rows consume ilv a1 -> permute rows; first 512 cols -> ilv
    Wc2 = Wcat2[PERM, :]
    Wc2 = np.concatenate([Wc2[:, PERM], Wc2[:, FIN:]], axis=1)
    # Wcat3: rows consume ilv a2; outputs plain (H=1)
    Wc3 = Wcat3[PERM, :]

    shared = dict(
        rhs0=RHS0.astype(NP_BF16),
        w2=Wc2.astype(NP_BF16),
        w3=Wc3.astype(NP_BF16),
        encb=encb_ilv.astype(NP_BF16)[None, :],
        eb1=eb1.astype(NP_BF16),
        g1=np.asarray(inputs['g1'], np.float32)[PERM][None, :],
        be1=np.asarray(inputs['be1'], np.float32)[PERM][None, :],
        g2=np.asarray(inputs['g2'], np.float32)[PERM][None, :],
        be2=np.asarray(inputs['be2'], np.float32)[PERM][None, :],
        g1T=np.ascontiguousarray(
            np.asarray(inputs['g1'], np.float32)[PERM].reshape(4, 128).T),
        be1T=np.ascontiguousarray(
            np.asarray(inputs['be1'], np.float32)[PERM].reshape(4, 128).T),
        g2T=np.ascontiguousarray(
            np.asarray(inputs['g2'], np.float32)[PERM].reshape(4, 128).T),
        be2T=np.ascontiguousarray(
            np.asarray(inputs['be2'], np.float32)[PERM].reshape(4, 128).T),
        g3=np.asarray(inputs['g3'], np.float32)[None, :],
        be3=np.asarray(inputs['be3'], np.float32)[None, :],
        linW=np.asarray(inputs['lin_W'], np.float32),
        linb=np.asarray(inputs['lin_b'], np.float32)[:, None],
        ident=np.eye(128, dtype=np.float32),
        identb=np.eye(128, dtype=NP_BF16),
        indmat=np.broadcast_to((np.bincount(np.asarray(inputs['batch'],
            np.int64), minlength=G) > 0).astype(np.float32)[None, :],
            (C, G)).copy(),
    )
    return T.tolist(), TT, per_core, shared


def _build(T_list, TT, repeat=1):
    nc = bacc.Bacc(None, target_bir_lowering=False, debug=False, num_devices=P,
                   num_swdge_queues=2)
    NE = TT * 128
    sbase = [0] * NT
    for b in range(1, NT):
        sbase[b] = sbase[b - 1] + T_list[b - 1]
    TMAXB = max(T_list)

    # ---- external inputs ----
    S_d = nc.dram_tensor("S", [128, NE], FP8, kind="ExternalInput")
    ST_d = nc.dram_tensor("ST", [128, NE], FP8, kind="ExternalInput")
    gidx_d = nc.dram_tensor("gidx", [128, NE // 16], I16, kind="ExternalInput")
    x0T_d = nc.dram_tensor("x0T", [FIN, NT * 128], BF16, kind="ExternalInput")
    pool_d = nc.dram_tensor("pool", [NT, 128, G], F32, kind="ExternalInput")
    rhs0_d = nc.dram_tensor("rhs0", [FIN, 1040], BF16, kind="ExternalInput")
    w2_d = nc.dram_tensor("w2", [FIN, SS12], BF16, kind="ExternalInput")
    w3_d = nc.dram_tensor("w3", [FIN, SS3], BF16, kind="ExternalInput")
    encb_d = nc.dram_tensor("encb", [1, FIN], BF16, kind="ExternalInput")
    eb1_d = nc.dram_tensor("eb1", [1, SS12], BF16, kind="ExternalInput")
    bn_d = {}
    for ly, wd in ((1, FIN), (2, FIN), (3, C)):
        bn_d[ly] = (nc.dram_tensor(f"g{ly}", [1, wd], F32, kind="ExternalInput"),
                    nc.dram_tensor(f"be{ly}", [1, wd], F32, kind="ExternalInput"))
    bnT_d = {ly: (nc.dram_tensor(f"g{ly}T", [128, 4], F32, kind="ExternalInput"),
                  nc.dram_tensor(f"be{ly}T", [128, 4], F32, kind="ExternalInput"))
             for ly in (1, 2)}
    linW_d = nc.dram_tensor("linW", [C, NCLS], F32, kind="ExternalInput")
    linb_d = nc.dram_tensor("linb", [NCLS, 1], F32, kind="ExternalInput")
    ident_d = nc.dram_tensor("ident", [128, 128], F32, kind="ExternalInput")
    identb_d = nc.dram_tensor("identb", [128, 128], BF16, kind="ExternalInput")
    indmat_d = nc.dram_tensor("indmat", [C, G], F32, kind="ExternalInput")
    out_d = nc.dram_tensor("out", [G, NCLS], F32, kind="ExternalOutput")

    # ---- internal DRAM ----
    cc_in = {1: nc.dram_tensor("cc_in1", [NL, ROW12], BF16),
             2: nc.dram_tensor("cc_in2", [NL, ROW12], BF16),
             3: nc.dram_tensor("cc_in3", [NL, ROW3], BF16)}
    cc_out = {1: nc.dram_tensor("cc_out1", [N, ROW12], BF16, addr_space="Shared"),
              2: nc.dram_tensor("cc_out2", [N, ROW12], BF16, addr_space="Shared"),
              3: nc.dram_tensor("cc_out3", [N, ROW3], BF16, addr_space="Shared")}
    st_in = {1: nc.dram_tensor("st_in1", [128, 8], F32),
             2: nc.dram_tensor("st_in2", [128, 8], F32)}
    st_out = {1: nc.dram_tensor("st_out1", [P * 128, 8], F32, addr_space="Shared"),
              2: nc.dram_tensor("st_out2", [P * 128, 8], F32, addr_space="Shared")}
    ar3_in = nc.dram_tensor("ar3_in", [C + 2, G], F32)
    ar3_out = nc.dram_tensor("ar3_out", [(C + 2) * P, G], F32, addr_space="Shared")
    RG = [list(range(P))]

    with tile.TileContext(nc) as tc:
        with tc.tile_pool(name="cn", bufs=1) as cn, \
             tc.tile_pool(name="xb", bufs=1) as xb, \
             tc.tile_pool(name="gp", bufs=2) as gp, \
             tc.tile_pool(name="wp", bufs=2) as wp, \
             tc.tile_pool(name="sm", bufs=2) as sm, \
             tc.tile_pool(name="psA", bufs=2, space="PSUM") as psA, \
             tc.tile_pool(name="psB", bufs=1, space="PSUM") as psB, \
             tc.tile_pool(name="psU", bufs=2, space="PSUM") as psU:

            # ---- load constants ----
            def cload(name, shape, dtype, dram, rearr=None, eng=None,
                      bufs=None, **kw):
                t = cn.tile(shape, dtype, tag=name, bufs=bufs)
                src = dram[:] if rearr is None else dram[:].rearrange(rearr, **kw)
                (eng or nc.gpsimd).dma_start(t[:], src)
                return t

            idx_sb = cload("idx", [128, NE // 16], I16, gidx_d)
            pool_sb = cload("pool", [128, NT, G], F32, pool_d, "n p g -> p n g")
            ident_sb = cload("ident", [128, 128], F32, ident_d)
            identb_sb = cload("identb", [128, 128], BF16, identb_d)
            encb_sb = cload("encb", [1, FIN], BF16, encb_d, eng=nc.sync)
            eb1_sb = cload("eb1", [1, SS12], BF16, eb1_d, eng=nc.sync)
            w3_sb = cload("w3", [128, 4, SS3], BF16, w3_d, "(k p) x -> p k x", p=128)
            linW_sb = cload("linW", [C, NCLS], F32, linW_d)
            indmat_sb = cload("indmat", [C, G], F32, indmat_d)
            linb_sb = cload("linb", [NCLS, 1], F32, linb_d)
            bn_sb = {3: (cload("g3", [1, C], F32, bn_d[3][0], bufs=1),
                          cload("be3", [1, C], F32, bn_d[3][1], bufs=1))}
            bnT_sb = {ly: (cload(f"g{ly}T", [128, 4], F32, bnT_d[ly][0],
                           bufs=1),
                           cload(f"be{ly}T", [128, 4], F32, bnT_d[ly][1],
                           bufs=1))
                      for ly in (1, 2)}
            # resident one-hot matrices (all 3 layers)
            S_res = cn.tile([128, TT, 128], FP8, tag="S_res")
            ST_res = cn.tile([128, TT, 128], FP8, tag="ST_res")
            half = (TT // 2) * 128
            nc.gpsimd.dma_start(S_res[:, 0:TT // 2, :], S_d[:, 0:half])
            nc.scalar.dma_start(S_res[:, TT // 2:TT, :], S_d[:, half:NE])
            nc.gpsimd.dma_start(ST_res[:, 0:TT // 2, :], ST_d[:, 0:half])
            nc.scalar.dma_start(ST_res[:, TT // 2:TT, :], ST_d[:, half:NE])

            ones_c = cn.tile([128, 1], BF16, tag="ones_c")
            nc.vector.memset(ones_c[:], 1.0)
            ones_cf = cn.tile([128, 1], F32, tag="ones_cf")
            nc.vector.memset(ones_cf[:], 1.0)
            ones_r = cn.tile([1, 128], BF16, tag="ones_r")
            nc.vector.memset(ones_r[:], 1.0)
            eb1bc = cn.tile([128, SS12], BF16, tag="eb1bc")
            nc.gpsimd.partition_broadcast(eb1bc[:], eb1_sb[:])
            encbc = cn.tile([128, FIN], BF16, tag="encbc")
            nc.gpsimd.partition_broadcast(encbc[:], encb_sb[:])
            epsbn_t = cn.tile([1, 1], F32, tag="epsbn")
            nc.vector.memset(epsbn_t[:], EPS_BN)
            zeros_c = cn.tile([128, 1], BF16, tag="zeros_c")
            nc.vector.memset(zeros_c[:], 0.0)

            # big rotating node-feature buffers (bf16)
            bufs = [xb.tile([128, NT, FIN], BF16, tag=f"big{i}", name=f"big{i}")
                    for i in range(3)]

            def nvalid(n):
                return 128 if n < NT - 1 else LAST

            # ---------- h_ext matmul phase ----------
            def h_phase(ly, lhsT_sb, wcat_sb, wofs, ss, bias_sb, xe_buf, sdloc):
                for n in range(NT):
                    ht = sm.tile([128, SS12], BF16, tag="hrow")
                    p5 = psA.tile([128, FIN], F32, tag="mm5")
                    pS = psB.tile([128, SS3], F32, tag="Z", name="pS")
                    for k in range(4):
                        lt = lhsT_sb[:, k, 128 * n:128 * (n + 1)]
                        if ly < 3:
                            nc.tensor.matmul(p5[:], lt,
                                             wcat_sb[:, k, wofs:wofs + FIN],
                                             start=(k == 0), stop=(k == 3))
                            nc.tensor.matmul(pS[:, 0:16], lt,
                                             wcat_sb[:, k, wofs + FIN:wofs + ss],
                                             start=(k == 0), stop=(k == 3))
                        else:
                            nc.tensor.matmul(pS[:, 0:SS3], lt,
                                             wcat_sb[:, k, 0:SS3],
                                             start=(k == 0), stop=(k == 3))
                    if ly == 1:
                        nc.vector.tensor_tensor(out=ht[:, 0:FIN], in0=p5[:],
                                                in1=eb1bc[:, 0:FIN], op=AL.add)
                        nc.vector.tensor_tensor(out=ht[:, FIN:ss],
                                                in0=pS[:, 0:16],
                                                in1=eb1bc[:, FIN:ss], op=AL.add)
                        nc.vector.tensor_copy(sdloc[:, n, :], ht[:, 520:528])
                    elif ly == 2:
                        nc.scalar.copy(ht[:, 0:FIN], p5[:])
                        nc.scalar.copy(ht[:, FIN:ss], pS[:, 0:16])
                        nc.scalar.copy(sdloc[:, n, :], pS[:, 8:16])
                    else:
                        nc.scalar.copy(ht[:, 0:SS3], pS[:, 0:SS3])
                        nc.scalar.copy(sdloc[:, n, :], pS[:, 65:66])
                    v = nvalid(n)
                    nc.sync.dma_start(
                        cc_in[ly][128 * n:128 * n + v, 0:ss], ht[0:v, 0:ss])
                if ly == 1:
                    # xe (residual base) deprioritized: fills PE gaps during
                    # the edge phase
                    with tc.high_priority(offset=-500000):
                        for n in range(NT):
                            pxe = psA.tile([128, FIN], F32, tag="mm5", name="pxe")
                            for k in range(4):
                                nc.tensor.matmul(
                                    pxe[:], lhsT_sb[:, k, 128 * n:128 * (n + 1)],
                                    wcat_sb[:, k, 0:FIN], start=(k == 0),
                                    stop=(k == 3))
                            nc.vector.tensor_tensor(out=xe_buf[:, n, :],
                                                    in0=pxe[:], in1=encbc[:],
                                                    op=AL.add)

            # ---------- edge aggregation phase ----------
            gcnt = {"g": 0, "g3": 0}

            def edge_phase(ly, rowv, ss, nh, fh, sdloc, ybuf, pstA, pstB):
                cph = fh // nh
                CH = 8
                gtag = "g" if ly < 3 else "g3"
                for b in range(NT):
                    T = T_list[b]
                    s0 = sbase[b]
                    w_t = wp.tile([128, TMAXB, 8], BF16, tag="w_t")
                    pU = psU.tile([128, FIN], F32, tag="U")
                    pZ = psB.tile([128, 8], F32, tag="Z")
                    first = True
                    for c0 in range(0, T, CH):
                        nsl = min(CH, T - c0)
                        sg = s0 + c0
                        g = gp.tile([128, CH, ROW12 if ly < 3 else ROW3],
                                    BF16, tag=gtag, bufs=3)
                        nc.gpsimd.dma_gather(
                            g[:, 0:nsl, 0:rowv], cc_out[ly][:],
                            idx_sb[:, 8 * sg:8 * (sg + nsl)],
                            num_idxs=nsl * 128, num_idxs_reg=nsl * 128,
                            elem_size=rowv, queue_num=0)
                        psd = psB.tile([128, CH * 8], F32, tag="sd", bufs=2)
                        for t in range(nsl):
                            nc.tensor.matmul(
                                psd[:, t * nh:(t + 1) * nh],
                                ST_res[:, sg + t, :],
                                sdloc[:, b, :], start=True, stop=True)
                        lg = wp.tile([128, CH * 8], F32, tag="lg")
                        nc.vector.tensor_tensor(
                            out=lg[:, 0:nsl * nh],
                            in0=g[:, 0:nsl, fh:fh + nh],
                            in1=psd[:, 0:nsl * nh], op=AL.add)
                        nc.vector.scalar_tensor_tensor(
                            out=lg[:, 0:nsl * nh], in0=lg[:, 0:nsl * nh],
                            scalar=0.2, in1=lg[:, 0:nsl * nh],
                            op0=AL.mult, op1=AL.max)
                        nc.scalar.activation(
                            w_t[:, c0:c0 + nsl, 0:nh], lg[:, 0:nsl * nh],
                            ACTF.Exp)
                        # alpha-multiply: interleaved [c, h] layout -> packed
                        # last dim -> DVE 2x; alternate DVE/Pool for balance
                        eng_a = nc.vector
                        if nh == 8:
                            eng_a.tensor_tensor(
                                out=g[:, 0:nsl, 0:fh].rearrange(
                                    "p t (c h) -> p t c h", h=nh),
                                in0=g[:, 0:nsl, 0:fh].rearrange(
                                    "p t (c h) -> p t c h", h=nh),
                                in1=w_t[:, c0:c0 + nsl, :].unsqueeze(2)
                                    .broadcast_to([128, nsl, cph, nh]),
                                op=AL.mult)
                        else:
                            eng_a.tensor_tensor(
                                out=g[:, 0:nsl, 0:fh], in0=g[:, 0:nsl, 0:fh],
                                in1=w_t[:, c0:c0 + nsl, 0:nh].unsqueeze(3)
                                    .broadcast_to([128, nsl, nh, cph]),
                                op=AL.mult)
                        for t in range(nsl):
                            nc.tensor.matmul(
                                pU[:, 0:fh], S_res[:, sg + t, :],
                                g[:, t, 0:fh],
                                start=first, stop=(c0 + t == T - 1),
                                skip_group_check=True)
                            nc.tensor.matmul(
                                pZ[:, 0:nh], S_res[:, sg + t, :],
                                w_t[:, c0 + t, 0:nh],
                                start=first, stop=(c0 + t == T - 1),
                                skip_group_check=True)
                            first = False
                    rz = sm.tile([128, 8], F32, tag="rz")
                    nc.vector.tensor_scalar_add(rz[:, 0:nh], pZ[:, 0:nh], EPS_Z)
                    nc.vector.reciprocal(rz[:, 0:nh], rz[:, 0:nh])
                    if nh == 8:
                        nc.vector.tensor_tensor(
                            out=ybuf[:, b, 0:fh].rearrange(
                                "p (c h) -> p c h", h=nh),
                            in0=pU[:, 0:fh].rearrange("p (c h) -> p c h", h=nh),
                            in1=rz[:, 0:nh].unsqueeze(1)
                                .broadcast_to([128, cph, nh]),
                            op=AL.mult)
                    else:
                        nc.vector.tensor_tensor(
                            out=ybuf[:, b, 0:fh], in0=pU[:, 0:fh],
                            in1=rz[:, 0:nh].unsqueeze(2)
                                .broadcast_to([128, nh, cph]),
                            op=AL.mult)
                    y2 = sm.tile([128, FIN], BF16, tag="y2")
                    nc.scalar.activation(y2[:, 0:fh], ybuf[:, b, 0:fh],
                                         ACTF.Square)
                    if nh == 8:
                        if b == 0:
                            for kk in range(8):
                                nc.tensor.matmul(
                                    pstA[:, kk:kk + 1], identb_sb[:],
                                    zeros_c[:], start=True, stop=False,
                                    skip_group_check=True)
                        for k in range(4):
                            nc.tensor.matmul(
                                pstA[:, k:k + 1],
                                ybuf[:, b, 128 * k:128 * (k + 1)], ones_c[:],
                                start=False, stop=(b == NT - 1),
                                skip_group_check=True)
                            nc.tensor.matmul(
                                pstA[:, 4 + k:5 + k],
                                y2[:, 128 * k:128 * (k + 1)], ones_c[:],
                                start=False, stop=(b == NT - 1),
                                skip_group_check=True)
                    else:
                        nc.tensor.matmul(pstA[:, 0:fh], ones_cf[:],
                                         ybuf[:, b, 0:fh], start=(b == 0),
                                         stop=(b == NT - 1),
                                         skip_group_check=True)
                        nc.tensor.matmul(pstB[:, 0:fh], ones_c[:], y2[:, 0:fh],
                                         start=(b == 0), stop=(b == NT - 1),
                                         skip_group_check=True)

            # ---------- BN + (ELU + residual) ----------
            def bn_chain(ly, fh, ybuf, xprev, xnext, pst1, pst2):
                # transposed stats: pst1 [128, 8] = [sum(y) cols 0:4 | sum(y^2) 4:8]
                gT_sb, beT_sb = bnT_sb[ly]
                statT = sm.tile([128, 8], F32, tag="statT", bufs=1)
                nc.vector.tensor_copy(statT[:], pst1[:, 0:8])
                nc.sync.dma_start(st_in[ly][:], statT[:])
                nc.gpsimd.collective_compute(
                    "AllGather", AL.bypass, replica_groups=RG,
                    ins=[st_in[ly][:]], outs=[st_out[ly][:]])
                st8 = sm.tile([128, P, 8], F32, tag="st8", bufs=1)
                nc.sync.dma_start(
                    st8[:], st_out[ly][:].rearrange("(r p) c -> p r c", p=128))
                ss = sm.tile([128, 8], F32, tag="sstat", bufs=1)
                nc.vector.tensor_reduce(
                    out=ss[:], in_=st8[:].rearrange("p r c -> p c r"),
                    axis=AX.X, op=AL.add)
                mu = sm.tile([128, 4], F32, tag="muT", bufs=1)
                isd = sm.tile([128, 4], F32, tag="isdT", bufs=1)
                nc.vector.tensor_scalar_mul(mu[:], ss[:, 0:4], 1.0 / N)
                nc.vector.tensor_scalar_mul(ss[:, 4:8], ss[:, 4:8], 1.0 / N)
                nc.vector.tensor_tensor(out=isd[:], in0=mu[:], in1=mu[:],
                                        op=AL.mult)
                nc.vector.tensor_tensor(out=isd[:], in0=ss[:, 4:8],
                                        in1=isd[:], op=AL.subtract)
                nc.vector.tensor_scalar_add(isd[:], isd[:], EPS_BN)
                nc.scalar.activation(isd[:], isd[:], ACTF.Ln)
                nc.vector.tensor_scalar_mul(isd[:], isd[:], -0.5)
                nc.scalar.activation(isd[:], isd[:], ACTF.Exp)
                scfT = sm.tile([128, 4], F32, tag="scfT", bufs=1)
                shfT = sm.tile([128, 4], F32, tag="shfT", bufs=1)
                nc.vector.tensor_tensor(out=scfT[:], in0=gT_sb[:],
                                        in1=isd[:], op=AL.mult)
                nc.vector.tensor_tensor(out=shfT[:], in0=scfT[:],
                                        in1=mu[:], op=AL.mult)
                nc.vector.tensor_tensor(out=shfT[:], in0=beT_sb[:],
                                        in1=shfT[:], op=AL.subtract)
                # scf/shf [128,4] -> per-col rows [1,128] -> bcast [128, 512]
                psc2 = psB.tile([1, 4, 128], F32, tag="sd", bufs=2,
                                name="psc2")
                psc3 = psB.tile([1, 4, 128], F32, tag="sd", bufs=2,
                                name="psc3")
                for k in range(4):
                    nc.tensor.transpose(psc2[:, k, :], scfT[:, k:k + 1],
                                        ident_sb[:])
                    nc.tensor.transpose(psc3[:, k, :], shfT[:, k:k + 1],
                                        ident_sb[:])
                row4 = sm.tile([1, 8, 128], F32, tag="row4", bufs=1)
                nc.vector.tensor_copy(row4[:, 0:4, :], psc2[:])
                nc.vector.tensor_copy(row4[:, 4:8, :], psc3[:])
                scT = sm.tile([128, FIN], F32, tag="scT", bufs=1)
                shT = sm.tile([128, FIN], F32, tag="shT", bufs=1)
                for k in range(4):
                    nc.gpsimd.partition_broadcast(
                        scT[:, 128 * k:128 * (k + 1)], row4[:, k, :])
                    nc.gpsimd.partition_broadcast(
                        shT[:, 128 * k:128 * (k + 1)], row4[:, 4 + k, :])
                for n in range(NT):
                    eng = nc.vector
                    eng1 = nc.gpsimd
                    v = sm.tile([128, FIN], BF16, tag="cht", name="v")
                    eng1.tensor_tensor(out=v[:, 0:fh],
                                       in0=ybuf[:, n, 0:fh],
                                       in1=scT[:, 0:fh], op=AL.mult)
                    eng1.tensor_tensor(out=v[:, 0:fh], in0=v[:, 0:fh],
                                       in1=shT[:, 0:fh], op=AL.add)
                    if ly == 3:
                        eng.tensor_copy(xnext[:, n, 0:fh], v[:, 0:fh])
                        continue
                    m = sm.tile([128, FIN], BF16, tag="che", name="m")
                    eng.tensor_scalar_min(m[:, 0:fh], v[:, 0:fh], 0.0)
                    nc.scalar.activation(m[:, 0:fh], m[:, 0:fh], ACTF.Exp)
                    xm = sm.tile([128, FIN], BF16, tag="chx", name="xm")
                    eng.tensor_tensor(out=xm[:, 0:fh], in0=m[:, 0:fh],
                                      in1=xprev[:, n, 0:fh], op=AL.add)
                    eng.tensor_scalar_max(v[:, 0:fh], v[:, 0:fh], 0.0)
                    eng.scalar_tensor_tensor(
                        out=xnext[:, n, 0:fh], in0=v[:, 0:fh], scalar=-1.0,
                        in1=xm[:, 0:fh], op0=AL.add, op1=AL.add)

            # ---------- transpose a -> aT (bf16) ----------
            def transpose_phase(abuf, aT):
                for n in range(NT):
                    for k in range(4):
                        tr = psB.tile([128, 128], BF16, tag="sd", bufs=2, name="tr")
                        nc.tensor.transpose(
                            tr[:], abuf[:, n, 128 * k:128 * (k + 1)],
                            identb_sb[:])
                        if k % 2:
                            nc.scalar.copy(aT[:, k, 128 * n:128 * (n + 1)], tr[:])
                        else:
                            nc.vector.tensor_copy(aT[:, k, 128 * n:128 * (n + 1)], tr[:])

            # =========== emit program ===========
            for _rep in range(repeat):
              xe, ybuf1, a1 = bufs[0], bufs[1], bufs[2]
              x0T_sb = xb.tile([128, 4, NT * 128], BF16, tag="lhsT",
                               name="x0T_sb")
              nc.sync.dma_start(x0T_sb[:],
                                x0T_d[:].rearrange("(k p) x -> p k x", p=128))
              sdloc = xb.tile([128, NT, 8], BF16, tag="sdloc", name="sdloc")
              wcat0 = cn.tile([128, 4, 1040], BF16, tag="wcat")
              nc.scalar.dma_start(wcat0[:], rhs0_d[:].rearrange("(k p) x -> p k x", p=128))

              # encoder + L1 h
              h_phase(1, x0T_sb, wcat0, FIN, SS12, eb1_sb, xe, sdloc)
              nc.gpsimd.collective_compute(
                  "AllGather", AL.bypass, replica_groups=RG,
                  ins=[cc_in[1][:]], outs=[cc_out[1][:]])
              pstA1 = psB.tile([128, 8], F32, tag="pstT", bufs=1, name="pstA1")
              edge_phase(1, ROW12, SS12, H, FIN, sdloc, ybuf1, pstA1, None)
              bn_chain(1, FIN, ybuf1, xe, a1, pstA1, None)

              # L2: a1 -> aT, h, edges (reuse xe buf as ybuf2, ybuf1 as a2)
              aT = xb.tile([128, 4, NT * 128], BF16, tag="lhsT")
              transpose_phase(a1, aT)
              wcat2 = cn.tile([128, 4, SS12], BF16, tag="wcat")
              nc.sync.dma_start(wcat2[:], w2_d[:].rearrange("(k p) x -> p k x", p=128))
              sdloc2 = xb.tile([128, NT, 8], BF16, tag="sdloc")
              h_phase(2, aT, wcat2, 0, SS12, None, None, sdloc2)
              nc.gpsimd.collective_compute(
                  "AllGather", AL.bypass, replica_groups=RG,
                  ins=[cc_in[2][:]], outs=[cc_out[2][:]])
              ybuf2, a2 = xe, ybuf1
              pstA2 = psB.tile([128, 8], F32, tag="pstT", bufs=1, name="pstA2")
              edge_phase(2, ROW12, SS12, H, FIN, sdloc2, ybuf2, pstA2, None)
              bn_chain(2, FIN, ybuf2, a1, a2, pstA2, None)

              # L3
              aT2 = xb.tile([128, 4, NT * 128], BF16, tag="lhsT")
              transpose_phase(a2, aT2)
              sdloc3 = xb.tile([128, NT, 1], BF16, tag="sdloc")
              h_phase(3, aT2, w3_sb, 0, SS3, None, None, sdloc3)
              nc.gpsimd.collective_compute(
                  "AllGather", AL.bypass, replica_groups=RG,
                  ins=[cc_in[3][:]], outs=[cc_out[3][:]])
              y3 = xb.tile([128, NT, C], F32, tag="y3")
              pstA3 = psA.tile([1, FIN], F32, tag="mm5", name="pstA3")
              pstB3 = psA.tile([1, FIN], F32, tag="mm5", name="pstB3")
              edge_phase(3, ROW3, SS3, 1, C, sdloc3, y3, pstA3, pstB3)

              # L3 stats + pooled sums, one AllGather for both
              stat3 = sm.tile([1, 2 * FIN], F32, tag="stat", name="stat3", bufs=1)
              nc.scalar.copy(stat3[:, 0:C], pstA3[:, 0:C])
              nc.scalar.copy(stat3[:, C:2 * C], pstB3[:, 0:C])
              # pooling on pre-BN y3: ygT[f, g] = sum_n y3[n, f] pool[n, g]
              pxg = psB.tile([C, G], F32, tag="sd", bufs=2, name="pxg")
              for n in range(NT):
                  nc.tensor.matmul(pxg[:], y3[:, n, :], pool_sb[:, n, :],
                                   start=(n == 0), stop=(n == NT - 1))
              xg = sm.tile([C, G], F32, tag="xg")
              nc.scalar.copy(xg[:], pxg[:])
              nc.sync.dma_start(ar3_in[0:C, :], xg[:])
              nc.sync.dma_start(ar3_in[C:C + 1, :], stat3[:, 0:C])
              nc.sync.dma_start(ar3_in[C + 1:C + 2, :], stat3[:, C:2 * C])
              nc.gpsimd.collective_compute(
                  "AllGather", AL.bypass, replica_groups=RG,
                  ins=[ar3_in[:]], outs=[ar3_out[:]])
              pooled8 = sm.tile([C, P, G], F32, tag="pooled8", bufs=1)
              nc.sync.dma_start(
                  pooled8[:, :, :],
                  ar3_out[:].rearrange("(r i) g -> i r g", r=P)[0:C])
              yg2 = sm.tile([C, G], F32, tag="xg2")
              nc.vector.tensor_reduce(
                  out=yg2[:, :],
                  in_=pooled8[:, :, :].rearrange("i r g -> i g r"),
                  axis=AX.X, op=AL.add)
              st8b = sm.tile([P, 2 * C], F32, tag="st8", bufs=1, name="st8b")
              nc.sync.dma_start(
                  st8b[:, :],
                  ar3_out[:].rearrange("(r i) g -> r (i g)", r=P)
                  [:, C * G:C * G + 2 * C])
              pm3 = psA.tile([1, FIN], F32, tag="mm5", name="pm3")
              nc.tensor.matmul(pm3[:, 0:2 * C], ones_cf[0:P, :],
                               st8b[:, :], start=True, stop=True)
              st3 = sm.tile([1, 2 * FIN], F32, tag="stat2", name="st3", bufs=1)
              nc.scalar.copy(st3[:, 0:2 * C], pm3[:, 0:2 * C])
              mu3 = st3[:, 0:C]
              ex23 = st3[:, C:2 * C]
              nc.vector.tensor_scalar_mul(mu3, mu3, 1.0 / N)
              nc.vector.tensor_scalar_mul(ex23, ex23, 1.0 / N)
              var3 = sm.tile([1, FIN], F32, tag="var", name="var3", bufs=1)
              nc.vector.tensor_tensor(out=var3[:, 0:C], in0=mu3, in1=mu3,
                                      op=AL.mult)
              nc.vector.tensor_tensor(out=var3[:, 0:C], in0=ex23,
                                      in1=var3[:, 0:C], op=AL.subtract)
              sd3 = sm.tile([1, FIN], F32, tag="sdv", name="sd3", bufs=1)
              nc.vector.tensor_scalar_add(var3[:, 0:C], var3[:, 0:C], EPS_BN)
              nc.scalar.activation(sd3[:, 0:C], var3[:, 0:C], ACTF.Ln)
              nc.vector.tensor_scalar_mul(sd3[:, 0:C], sd3[:, 0:C], -0.5)
              nc.scalar.activation(sd3[:, 0:C], sd3[:, 0:C], ACTF.Exp)
              g3_sb, be3_sb = bn_sb[3]
              scf3 = sm.tile([1, FIN], F32, tag="scf", name="scf3", bufs=1)
              nc.vector.tensor_tensor(out=scf3[:, 0:C], in0=g3_sb[:],
                                      in1=sd3[:, 0:C], op=AL.mult)
              shf3 = sm.tile([1, FIN], F32, tag="shf", name="shf3", bufs=1)
              nc.vector.tensor_tensor(out=shf3[:, 0:C], in0=scf3[:, 0:C],
                                      in1=mu3, op=AL.mult)
              nc.vector.tensor_tensor(out=shf3[:, 0:C], in0=be3_sb[:],
                                      in1=shf3[:, 0:C], op=AL.subtract)
              # transpose scf3/shf3 rows into per-partition columns [C, 1]
              psc = psB.tile([C, 1], F32, tag="Z", name="psc")
              nc.tensor.transpose(psc[:], scf3[:, 0:C], ident_sb[0:1, 0:1])
              scol = sm.tile([C, 1], F32, tag="scol", name="scol")
              nc.scalar.copy(scol[:], psc[:])
              psh = psB.tile([C, 1], F32, tag="Z", name="psh")
              nc.tensor.transpose(psh[:], shf3[:, 0:C], ident_sb[0:1, 0:1])
              shcol = sm.tile([C, 1], F32, tag="shcol", name="shcol")
              nc.scalar.copy(shcol[:], psh[:])
              # xgbn = yg2 * scol + shcol * indmat
              sh_t = sm.tile([C, G], F32, tag="shterm", name="sh_t")
              nc.vector.tensor_scalar_mul(sh_t[:], indmat_sb[:], shcol[:])
              xgbn = sm.tile([C, G], F32, tag="xgbn", name="xgbn")
              nc.vector.scalar_tensor_tensor(
                  out=xgbn[:], in0=yg2[:], scalar=scol[:], in1=sh_t[:],
                  op0=AL.mult, op1=AL.add)
              # outT[nc, g] = linW.T @ xgbn  (contract over f)
              pot = psB.tile([NCLS, G], F32, tag="sd", bufs=2, name="pot")
              nc.tensor.matmul(pot[:], linW_sb[:], xgbn[:], start=True,
                               stop=True)
              outT = sm.tile([NCLS, G], F32, tag="outT")
              nc.scalar.activation(outT[:], pot[:], ACTF.Identity,
                                   bias=linb_sb[:])
              pfin = psB.tile([G, NCLS], F32, tag="sd", bufs=2, name="pfin")
              nc.tensor.transpose(pfin[:], outT[:], ident_sb[0:NCLS, 0:NCLS])
              fin = sm.tile([G, NCLS], F32, tag="fin_sb")
              nc.vector.tensor_copy(fin[:], pfin[:])
              nc.sync.dma_start(out_d[:], fin[:])

        sched_state, snap = tc.schedule_and_allocate()
        nc._sched_state = sched_state
        nc._pred_ns = snap.time

    nc.finalize()
    return nc


_CACHE = {}


def _get_nc(T_key, TT, repeat=1):
    key = (T_key, repeat)
    if key not in _CACHE:
        _CACHE[key] = _build(list(T_key), TT, repeat)
    return _CACHE[key]


def make_in_maps(per_core, shared):
    return [dict(S=pc['S'], ST=pc['ST'], gidx=pc['gidx'],
                 x0T=pc['x0T'], pool=pc['pool'], **shared)
            for pc in per_core]


def kernel(**inputs):
    T_list, TT, per_core, shared = _prep(inputs)
    nc = _get_nc(tuple(T_list), TT)
    in_maps = make_in_maps(per_core, shared)
    res = run_bass_kernel_spmd(nc, in_maps, core_ids=list(range(P)))
    return np.asarray(res.results[0]['out'], np.float32)


# revision 28
# speedup vs baseline: 1.1589x; 1.0017x over previous
"""Trainium2 Bass kernel: 3-layer GAT + BN + ELU + residual + global mean pool + linear.

Sharding: nodes (and their incident edges, grouped by destination) are
sharded across 8 NeuronCores. Weights replicated. Per layer:
  1. local h_ext = x_local @ [W | W@As | W@Ad]  (node-major rows)
  2. AllGather h_ext -> full [N, ROW] table in DRAM (bf16)
  3. per dst-block: dma_gather of h_ext[src] rows for this core's edges,
     attention weights w = exp(leaky(sS[src]+sD[dst])) via one-hot
     broadcast matmul; weighted scatter-matmul accumulates U and Z in
     PSUM; y = U/(Z+eps)
  4. BN stats (ones-matmul) -> AllGather -> scale/shift -> ELU -> residual
Pool + final linear at the end (AllGather of pooled sums).

Perf notes (cost-model driven):
  - 512-wide features stored interleaved [c, h] (c-major) so the per-edge
    attention multiply has a packed last dim -> DVE 2x mode. The
    interleave is a pure host-side permutation of weight rows/cols.
  - One-hot S (edge->dst) and ST matrices are bf16-resident in SBUF for
    all 3 layers (loaded once).
  - One dma_gather per dst block (18 slots) to amortize the SWDGE fixed
    overhead on the Pool engine.
  - BN statistics matmuls run on bf16 copies (4x cheaper on PE than f32).
  - alpha-multiply alternates DVE/Pool by block to balance engine load.
"""
import sys
if '/opt/trn_rl_repo' not in sys.path:
    sys.path.insert(0, '/opt/trn_rl_repo')
import numpy as np
import ml_dtypes

import concourse.bass as bass
import concourse.bacc as bacc
import concourse.mybir as mybir
from concourse import tile
from concourse.bass_utils import run_bass_kernel_spmd

F32 = mybir.dt.float32
FP8 = mybir.dt.float8e4
BF16 = mybir.dt.bfloat16
I16 = mybir.dt.int16
AL = mybir.AluOpType
ACTF = mybir.ActivationFunctionType
AX = mybir.AxisListType

N, E, FIN, H, C, G, NCLS = 10000, 160000, 512, 8, 64, 64, 64
P = 8
NL = N // P            # 1250 nodes per core
NT = 10                # node tiles per core (9x128 + 98)
LAST = NL - 9 * 128    # 98
ROW12 = 640            # bf16 gather row (640*2B = 1280B, %256==0); data in 0:528
ROW3 = 128             # bf16 gather row L3 (256B); data in 0:66
SS12 = 528             # h(512 ilv) | sS(8) | sD(8)
SS3 = 66               # h(64) | sS(1) | sD(1)
EPS_Z = 1e-16
EPS_BN = 1e-5
NP_BF16 = ml_dtypes.bfloat16
NP_FP8 = ml_dtypes.float8_e4m3

# interleave permutation: ilv position c*8+h  <- std position h*64+c
PERM = np.arange(FIN).reshape(H, C).T.reshape(-1)   # PERM[c*8+h] = h*64+c


def _blockdiag(a):
    # a [H, C] -> [H*C, H] with column h holding a[h] in rows h*C:(h+1)*C
    hh, cc = a.shape
    out = np.zeros((hh * cc, hh), np.float64)
    for h in range(hh):
        out[h * cc:(h + 1) * cc, h] = a[h]
    return out


def _prep(inputs):
    x = np.asarray(inputs['x'], np.float32)
    ei = np.asarray(inputs['edge_index'], np.int64)
    batch = np.asarray(inputs['batch'], np.int64)

    src = np.concatenate([ei[0], np.arange(N, dtype=np.int64)])
    dst = np.concatenate([ei[1], np.arange(N, dtype=np.int64)])
    order = np.argsort(dst, kind='stable')
    src, dst = src[order], dst[order]

    core = dst // NL
    blk = (dst % NL) // 128
    dloc = (dst % NL) % 128

    per_cb = {}
    T = np.ones(NT, np.int64)
    for c in range(P):
        m = core == c
        sc, dc, bc = src[m], dloc[m], blk[m]
        for b in range(NT):
            mb = bc == b
            per_cb[(c, b)] = (sc[mb], dc[mb])
            T[b] = max(T[b], (int(mb.sum()) + 127) // 128)
    sbase = np.zeros(NT, np.int64)
    sbase[1:] = np.cumsum(T)[:-1]
    TT = int(T.sum())
    NE = TT * 128

    per_core = []
    for c in range(P):
        sidx = np.zeros(NE, np.int64)
        dl = np.full(NE, 255, np.int64)
        for b in range(NT):
            sc, dc = per_cb[(c, b)]
            off = int(sbase[b]) * 128
            sidx[off:off + len(sc)] = sc
            dl[off:off + len(dc)] = dc
        j = np.arange(NE)
        t, pp = j // 128, j % 128
        valid = dl < 128
        S = np.zeros((TT, 128, 128), NP_FP8)
        S[t[valid], pp[valid], dl[valid]] = 1
        # resident layouts: [128, TT*128]
        S_flat = np.ascontiguousarray(S.transpose(1, 0, 2).reshape(128, TT * 128))
        ST_flat = np.ascontiguousarray(S.transpose(2, 0, 1).reshape(128, TT * 128))
        g16 = np.zeros((16, NE // 16), np.int16)
        g16[j % 16, j // 16] = sidx.astype(np.int16)
        gidx = np.tile(g16, (8, 1))

        xc = x[c * NL:(c + 1) * NL]                      # [1250, 512]
        x0T = np.zeros((FIN, NT * 128), np.float32)
        x0T[:, :NL] = xc.T
        x0T = x0T.astype(NP_BF16)

        cnt = np.bincount(batch, minlength=G).astype(np.float64)
        inv = 1.0 / np.maximum(cnt, 1.0)
        pool = np.zeros((NT, 128, G), np.float32)
        nodes = np.arange(NL) + c * NL
        nn, ppp = np.arange(NL) // 128, np.arange(NL) % 128
        pool[nn, ppp, batch[nodes]] = inv[batch[nodes]]

        per_core.append(dict(S=S_flat, ST=ST_flat, gidx=gidx, x0T=x0T,
                             pool=pool))

    f64 = lambda k: np.asarray(inputs[k], np.float64)
    W1, W2, W3 = f64('W1'), f64('W2'), f64('W3')
    # std-basis cat weights, then permute for the interleaved layout:
    #  - 512-wide activation streams (enc out, a1, a2, y1, y2) live in ilv
    #  - Wcat1 consumes std(enc raw in)=x@encW... enc out is ilv so W1 rows perm
    Wcat1 = np.concatenate(
        [W1, W1 @ _blockdiag(f64('as1')), W1 @ _blockdiag(f64('ad1'))], axis=1)
    Wcat2 = np.concatenate(
        [W2, W2 @ _blockdiag(f64('as2')), W2 @ _blockdiag(f64('ad2'))], axis=1)
    Wcat3 = np.concatenate(
        [W3, (W3 @ f64('as3')[0])[:, None], (W3 @ f64('ad3')[0])[:, None]],
        axis=1)
    encW = f64('enc_W')
    # encoder part of RHS0: output cols in ilv
    enc_ilv = encW[:, PERM]
    # h1 part: encW(std out) @ Wcat1(std in); first 512 output cols -> ilv
    part2 = encW @ Wcat1
    part2 = np.concatenate([part2[:, PERM], part2[:, FIN:]], axis=1)
    RHS0 = np.concatenate([enc_ilv, part2], axis=1)          # [512, 1040]
    eb1 = (f64('enc_b') @ Wcat1)
    eb1 = np.concatenate([eb1[PERM], eb1[FIN:]])[None, :]     # [1, 528]
    encb_ilv = np.asarray(inputs['enc_b'], np.float64)[PERM]
    # Wcat2: rows consume ilv a1 -> permute rows; first 512 cols -> ilv
    Wc2 = Wcat2[PERM, :]
    Wc2 = np.concatenate([Wc2[:, PERM], Wc2[:, FIN:]], axis=1)
    # Wcat3: rows consume ilv a2; outputs plain (H=1)
    Wc3 = Wcat3[PERM, :]

    shared = dict(
        rhs0=RHS0.astype(NP_BF16),
        w2=Wc2.astype(NP_BF16),
        w3=Wc3.astype(NP_BF16),
        encb=encb_ilv.astype(NP_BF16)[None, :],
        eb1=eb1.astype(NP_BF16),
        g1=np.asarray(inputs['g1'], np.float32)[PERM][None, :],
        be1=np.asarray(inputs['be1'], np.float32)[PERM][None, :],
        g2=np.asarray(inputs['g2'], np.float32)[PERM][None, :],
        be2=np.asarray(inputs['be2'], np.float32)[PERM][None, :],
        g1T=np.ascontiguousarray(
            np.asarray(inputs['g1'], np.float32)[PERM].reshape(4, 128).T),
        be1T=np.ascontiguousarray(
            np.asarray(inputs['be1'], np.float32)[PERM].reshape(4, 128).T),
        g2T=np.ascontiguousarray(
            np.asarray(inputs['g2'], np.float32)[PERM].reshape(4, 128).T),
        be2T=np.ascontiguousarray(
            np.asarray(inputs['be2'], np.float32)[PERM].reshape(4, 128).T),
        g3=np.asarray(inputs['g3'], np.float32)[None, :],
        be3=np.asarray(inputs['be3'], np.float32)[None, :],
        linW=np.asarray(inputs['lin_W'], np.float32),
        linb=np.asarray(inputs['lin_b'], np.float32)[:, None],
        ident=np.eye(128, dtype=np.float32),
        identb=np.eye(128, dtype=NP_BF16),
        indmat=np.broadcast_to((np.bincount(np.asarray(inputs['batch'],
            np.int64), minlength=G) > 0).astype(np.float32)[None, :],
            (C, G)).copy(),
    )
    return T.tolist(), TT, per_core, shared


def _build(T_list, TT, repeat=1):
    nc = bacc.Bacc(None, target_bir_lowering=False, debug=False, num_devices=P,
                   num_swdge_queues=2)
    NE = TT * 128
    sbase = [0] * NT
    for b in range(1, NT):
        sbase[b] = sbase[b - 1] + T_list[b - 1]
    TMAXB = max(T_list)

    # ---- external inputs ----
    S_d = nc.dram_tensor("S", [128, NE], FP8, kind="ExternalInput")
    ST_d = nc.dram_tensor("ST", [128, NE], FP8, kind="ExternalInput")
    gidx_d = nc.dram_tensor("gidx", [128, NE // 16], I16, kind="ExternalInput")
    x0T_d = nc.dram_tensor("x0T", [FIN, NT * 128], BF16, kind="ExternalInput")
    pool_d = nc.dram_tensor("pool", [NT, 128, G], F32, kind="ExternalInput")
    rhs0_d = nc.dram_tensor("rhs0", [FIN, 1040], BF16, kind="ExternalInput")
    w2_d = nc.dram_tensor("w2", [FIN, SS12], BF16, kind="ExternalInput")
    w3_d = nc.dram_tensor("w3", [FIN, SS3], BF16, kind="ExternalInput")
    encb_d = nc.dram_tensor("encb", [1, FIN], BF16, kind="ExternalInput")
    eb1_d = nc.dram_tensor("eb1", [1, SS12], BF16, kind="ExternalInput")
    bn_d = {}
    for ly, wd in ((1, FIN), (2, FIN), (3, C)):
        bn_d[ly] = (nc.dram_tensor(f"g{ly}", [1, wd], F32, kind="ExternalInput"),
                    nc.dram_tensor(f"be{ly}", [1, wd], F32, kind="ExternalInput"))
    bnT_d = {ly: (nc.dram_tensor(f"g{ly}T", [128, 4], F32, kind="ExternalInput"),
                  nc.dram_tensor(f"be{ly}T", [128, 4], F32, kind="ExternalInput"))
             for ly in (1, 2)}
    linW_d = nc.dram_tensor("linW", [C, NCLS], F32, kind="ExternalInput")
    linb_d = nc.dram_tensor("linb", [NCLS, 1], F32, kind="ExternalInput")
    ident_d = nc.dram_tensor("ident", [128, 128], F32, kind="ExternalInput")
    identb_d = nc.dram_tensor("identb", [128, 128], BF16, kind="ExternalInput")
    indmat_d = nc.dram_tensor("indmat", [C, G], F32, kind="ExternalInput")
    out_d = nc.dram_tensor("out", [G, NCLS], F32, kind="ExternalOutput")

    # ---- internal DRAM ----
    cc_in = {1: nc.dram_tensor("cc_in1", [NL, ROW12], BF16),
             2: nc.dram_tensor("cc_in2", [NL, ROW12], BF16),
             3: nc.dram_tensor("cc_in3", [NL, ROW3], BF16)}
    cc_out = {1: nc.dram_tensor("cc_out1", [N, ROW12], BF16, addr_space="Shared"),
              2: nc.dram_tensor("cc_out2", [N, ROW12], BF16, addr_space="Shared"),
              3: nc.dram_tensor("cc_out3", [N, ROW3], BF16, addr_space="Shared")}
    st_in = {1: nc.dram_tensor("st_in1", [128, 8], F32),
             2: nc.dram_tensor("st_in2", [128, 8], F32)}
    st_out = {1: nc.dram_tensor("st_out1", [P * 128, 8], F32, addr_space="Shared"),
              2: nc.dram_tensor("st_out2", [P * 128, 8], F32, addr_space="Shared")}
    ar3_in = nc.dram_tensor("ar3_in", [C + 2, G], F32)
    ar3_out = nc.dram_tensor("ar3_out", [(C + 2) * P, G], F32, addr_space="Shared")
    RG = [list(range(P))]

    with tile.TileContext(nc) as tc:
        with tc.tile_pool(name="cn", bufs=1) as cn, \
             tc.tile_pool(name="xb", bufs=1) as xb, \
             tc.tile_pool(name="gp", bufs=2) as gp, \
             tc.tile_pool(name="wp", bufs=2) as wp, \
             tc.tile_pool(name="sm", bufs=2) as sm, \
             tc.tile_pool(name="psA", bufs=2, space="PSUM") as psA, \
             tc.tile_pool(name="psB", bufs=1, space="PSUM") as psB, \
             tc.tile_pool(name="psU", bufs=2, space="PSUM") as psU:

            # ---- load constants ----
            def cload(name, shape, dtype, dram, rearr=None, eng=None,
                      bufs=None, **kw):
                t = cn.tile(shape, dtype, tag=name, bufs=bufs)
                src = dram[:] if rearr is None else dram[:].rearrange(rearr, **kw)
                (eng or nc.gpsimd).dma_start(t[:], src)
                return t

            idx_sb = cload("idx", [128, NE // 16], I16, gidx_d)
            pool_sb = cload("pool", [128, NT, G], F32, pool_d, "n p g -> p n g")
            ident_sb = cload("ident", [128, 128], F32, ident_d)
            identb_sb = cload("identb", [128, 128], BF16, identb_d)
            encb_sb = cload("encb", [1, FIN], BF16, encb_d, eng=nc.sync)
            eb1_sb = cload("eb1", [1, SS12], BF16, eb1_d, eng=nc.sync)
            w3_sb = cload("w3", [128, 4, SS3], BF16, w3_d, "(k p) x -> p k x", p=128)
            linW_sb = cload("linW", [C, NCLS], F32, linW_d)
            indmat_sb = cload("indmat", [C, G], F32, indmat_d)
            linb_sb = cload("linb", [NCLS, 1], F32, linb_d)
            bn_sb = {3: (cload("g3", [1, C], F32, bn_d[3][0], bufs=1),
                          cload("be3", [1, C], F32, bn_d[3][1], bufs=1))}
            bnT_sb = {ly: (cload(f"g{ly}T", [128, 4], F32, bnT_d[ly][0],
                           bufs=1),
                           cload(f"be{ly}T", [128, 4], F32, bnT_d[ly][1],
                           bufs=1))
                      for ly in (1, 2)}
            # resident one-hot matrices (all 3 layers)
            S_res = cn.tile([128, TT, 128], FP8, tag="S_res")
            ST_res = cn.tile([128, TT, 128], FP8, tag="ST_res")
            half = (TT // 2) * 128
            nc.gpsimd.dma_start(S_res[:, 0:TT // 2, :], S_d[:, 0:half])
            nc.scalar.dma_start(S_res[:, TT // 2:TT, :], S_d[:, half:NE])
            nc.gpsimd.dma_start(ST_res[:, 0:TT // 2, :], ST_d[:, 0:half])
            nc.scalar.dma_start(ST_res[:, TT // 2:TT, :], ST_d[:, half:NE])

            ones_c = cn.tile([128, 1], BF16, tag="ones_c")
            nc.vector.memset(ones_c[:], 1.0)
            ones_cf = cn.tile([128, 1], F32, tag="ones_cf")
            nc.vector.memset(ones_cf[:], 1.0)
            ones_r = cn.tile([1, 128], BF16, tag="ones_r")
            nc.vector.memset(ones_r[:], 1.0)
            eb1bc = cn.tile([128, SS12], BF16, tag="eb1bc")
            nc.gpsimd.partition_broadcast(eb1bc[:], eb1_sb[:])
            encbc = cn.tile([128, FIN], BF16, tag="encbc")
            nc.gpsimd.partition_broadcast(encbc[:], encb_sb[:])
            epsbn_t = cn.tile([1, 1], F32, tag="epsbn")
            nc.vector.memset(epsbn_t[:], EPS_BN)
            zeros_c = cn.tile([128, 1], BF16, tag="zeros_c")
            nc.vector.memset(zeros_c[:], 0.0)

            # big rotating node-feature buffers (bf16)
            bufs = [xb.tile([128, NT, FIN], BF16, tag=f"big{i}", name=f"big{i}")
                    for i in range(3)]

            def nvalid(n):
                return 128 if n < NT - 1 else LAST

            # ---------- h_ext matmul phase ----------
            def h_phase(ly, lhsT_sb, wcat_sb, wofs, ss, bias_sb, xe_buf, sdloc):
                for n in range(NT):
                    ht = sm.tile([128, SS12], BF16, tag="hrow")
                    p5 = psA.tile([128, FIN], F32, tag="mm5")
                    pS = psB.tile([128, SS3], F32, tag="Z", name="pS")
                    for k in range(4):
                        lt = lhsT_sb[:, k, 128 * n:128 * (n + 1)]
                        if ly < 3:
                            nc.tensor.matmul(p5[:], lt,
                                             wcat_sb[:, k, wofs:wofs + FIN],
                                             start=(k == 0), stop=(k == 3))
                            nc.tensor.matmul(pS[:, 0:16], lt,
                                             wcat_sb[:, k, wofs + FIN:wofs + ss],
                                             start=(k == 0), stop=(k == 3))
                        else:
                            nc.tensor.matmul(pS[:, 0:SS3], lt,
                                             wcat_sb[:, k, 0:SS3],
                                             start=(k == 0), stop=(k == 3))
                    if ly == 1:
                        nc.vector.tensor_tensor(out=ht[:, 0:FIN], in0=p5[:],
                                                in1=eb1bc[:, 0:FIN], op=AL.add)
                        nc.vector.tensor_tensor(out=ht[:, FIN:ss],
                                                in0=pS[:, 0:16],
                                                in1=eb1bc[:, FIN:ss], op=AL.add)
                        nc.vector.tensor_copy(sdloc[:, n, :], ht[:, 520:528])
                    elif ly == 2:
                        nc.scalar.copy(ht[:, 0:FIN], p5[:])
                        nc.scalar.copy(ht[:, FIN:ss], pS[:, 0:16])
                        nc.scalar.copy(sdloc[:, n, :], pS[:, 8:16])
                    else:
                        nc.scalar.copy(ht[:, 0:SS3], pS[:, 0:SS3])
                        nc.scalar.copy(sdloc[:, n, :], pS[:, 65:66])
                    v = nvalid(n)
                    nc.sync.dma_start(
                        cc_in[ly][128 * n:128 * n + v, 0:ss], ht[0:v, 0:ss])
                if ly == 1:
                    # xe (residual base) deprioritized: fills PE gaps during
                    # the edge phase
                    with tc.high_priority(offset=-500000):
                        for n in range(NT):
                            pxe = psA.tile([128, FIN], F32, tag="mm5", name="pxe")
                            for k in range(4):
                                nc.tensor.matmul(
                                    pxe[:], lhsT_sb[:, k, 128 * n:128 * (n + 1)],
                                    wcat_sb[:, k, 0:FIN], start=(k == 0),
                                    stop=(k == 3))
                            nc.vector.tensor_tensor(out=xe_buf[:, n, :],
                                                    in0=pxe[:], in1=encbc[:],
                                                    op=AL.add)

            # ---------- edge aggregation phase ----------
            gcnt = {"g": 0, "g3": 0}

            def edge_phase(ly, rowv, ss, nh, fh, sdloc, ybuf, pstA, pstB):
                cph = fh // nh
                CH = 8
                gtag = "g" if ly < 3 else "g3"
                # process the ragged block (NT-1) first so the final stats
                # tail rides on a tiny 2-slot chunk
                order = [NT - 1] + list(range(NT - 1))
                for pos, b in enumerate(order):
                    T = T_list[b]
                    s0 = sbase[b]
                    w_t = wp.tile([128, TMAXB, 8], BF16, tag="w_t")
                    pU = psU.tile([128, FIN], F32, tag="U")
                    pZ = psB.tile([128, 8], F32, tag="Z")
                    first = True
                    for c0 in range(0, T, CH):
                        nsl = min(CH, T - c0)
                        sg = s0 + c0
                        g = gp.tile([128, CH, ROW12 if ly < 3 else ROW3],
                                    BF16, tag=gtag, bufs=3)
                        nc.gpsimd.dma_gather(
                            g[:, 0:nsl, 0:rowv], cc_out[ly][:],
                            idx_sb[:, 8 * sg:8 * (sg + nsl)],
                            num_idxs=nsl * 128, num_idxs_reg=nsl * 128,
                            elem_size=rowv, queue_num=0)
                        psd = psB.tile([128, CH * 8], F32, tag="sd", bufs=2)
                        for t in range(nsl):
                            nc.tensor.matmul(
                                psd[:, t * nh:(t + 1) * nh],
                                ST_res[:, sg + t, :],
                                sdloc[:, b, :], start=True, stop=True)
                        lg = wp.tile([128, CH * 8], F32, tag="lg")
                        nc.vector.tensor_tensor(
                            out=lg[:, 0:nsl * nh],
                            in0=g[:, 0:nsl, fh:fh + nh],
                            in1=psd[:, 0:nsl * nh], op=AL.add)
                        nc.vector.scalar_tensor_tensor(
                            out=lg[:, 0:nsl * nh], in0=lg[:, 0:nsl * nh],
                            scalar=0.2, in1=lg[:, 0:nsl * nh],
                            op0=AL.mult, op1=AL.max)
                        nc.scalar.activation(
                            w_t[:, c0:c0 + nsl, 0:nh], lg[:, 0:nsl * nh],
                            ACTF.Exp)
                        # alpha-multiply: interleaved [c, h] layout -> packed
                        # last dim -> DVE 2x; alternate DVE/Pool for balance
                        eng_a = nc.vector
                        if nh == 8:
                            eng_a.tensor_tensor(
                                out=g[:, 0:nsl, 0:fh].rearrange(
                                    "p t (c h) -> p t c h", h=nh),
                                in0=g[:, 0:nsl, 0:fh].rearrange(
                                    "p t (c h) -> p t c h", h=nh),
                                in1=w_t[:, c0:c0 + nsl, :].unsqueeze(2)
                                    .broadcast_to([128, nsl, cph, nh]),
                                op=AL.mult)
                        else:
                            eng_a.tensor_tensor(
                                out=g[:, 0:nsl, 0:fh], in0=g[:, 0:nsl, 0:fh],
                                in1=w_t[:, c0:c0 + nsl, 0:nh].unsqueeze(3)
                                    .broadcast_to([128, nsl, nh, cph]),
                                op=AL.mult)
                        for t in range(nsl):
                            nc.tensor.matmul(
                                pU[:, 0:fh], S_res[:, sg + t, :],
                                g[:, t, 0:fh],
                                start=first, stop=(c0 + t == T - 1),
                                skip_group_check=True)
                            nc.tensor.matmul(
                                pZ[:, 0:nh], S_res[:, sg + t, :],
                                w_t[:, c0 + t, 0:nh],
                                start=first, stop=(c0 + t == T - 1),
                                skip_group_check=True)
                            first = False
                    rz = sm.tile([128, 8], F32, tag="rz")
                    nc.vector.tensor_scalar_add(rz[:, 0:nh], pZ[:, 0:nh], EPS_Z)
                    nc.vector.reciprocal(rz[:, 0:nh], rz[:, 0:nh])
                    if nh == 8:
                        nc.vector.tensor_tensor(
                            out=ybuf[:, b, 0:fh].rearrange(
                                "p (c h) -> p c h", h=nh),
                            in0=pU[:, 0:fh].rearrange("p (c h) -> p c h", h=nh),
                            in1=rz[:, 0:nh].unsqueeze(1)
                                .broadcast_to([128, cph, nh]),
                            op=AL.mult)
                    else:
                        nc.vector.tensor_tensor(
                            out=ybuf[:, b, 0:fh], in0=pU[:, 0:fh],
                            in1=rz[:, 0:nh].unsqueeze(2)
                                .broadcast_to([128, nh, cph]),
                            op=AL.mult)
                    y2 = sm.tile([128, FIN], BF16, tag="y2")
                    nc.scalar.activation(y2[:, 0:fh], ybuf[:, b, 0:fh],
                                         ACTF.Square)
                    if nh == 8:
                        if pos == 0:
                            for kk in range(8):
                                nc.tensor.matmul(
                                    pstA[:, kk:kk + 1], identb_sb[:],
                                    zeros_c[:], start=True, stop=False,
                                    skip_group_check=True)
                        for k in range(4):
                            nc.tensor.matmul(
                                pstA[:, k:k + 1],
                                ybuf[:, b, 128 * k:128 * (k + 1)], ones_c[:],
                                start=False, stop=(pos == NT - 1),
                                skip_group_check=True)
                            nc.tensor.matmul(
                                pstA[:, 4 + k:5 + k],
                                y2[:, 128 * k:128 * (k + 1)], ones_c[:],
                                start=False, stop=(pos == NT - 1),
                                skip_group_check=True)
                    else:
                        nc.tensor.matmul(pstA[:, 0:fh], ones_cf[:],
                                         ybuf[:, b, 0:fh], start=(pos == 0),
                                         stop=(pos == NT - 1),
                                         skip_group_check=True)
                        nc.tensor.matmul(pstB[:, 0:fh], ones_c[:], y2[:, 0:fh],
                                         start=(pos == 0), stop=(pos == NT - 1),
                                         skip_group_check=True)

            # ---------- BN + (ELU + residual) ----------
            def bn_chain(ly, fh, ybuf, xprev, xnext, pst1, pst2):
                # transposed stats: pst1 [128, 8] = [sum(y) cols 0:4 | sum(y^2) 4:8]
                gT_sb, beT_sb = bnT_sb[ly]
                statT = sm.tile([128, 8], F32, tag="statT", bufs=1)
                nc.vector.tensor_copy(statT[:], pst1[:, 0:8])
                nc.sync.dma_start(st_in[ly][:], statT[:])
                nc.gpsimd.collective_compute(
                    "AllGather", AL.bypass, replica_groups=RG,
                    ins=[st_in[ly][:]], outs=[st_out[ly][:]])
                st8 = sm.tile([128, P, 8], F32, tag="st8", bufs=1)
                nc.sync.dma_start(
                    st8[:], st_out[ly][:].rearrange("(r p) c -> p r c", p=128))
                ss = sm.tile([128, 8], F32, tag="sstat", bufs=1)
                nc.vector.tensor_reduce(
                    out=ss[:], in_=st8[:].rearrange("p r c -> p c r"),
                    axis=AX.X, op=AL.add)
                mu = sm.tile([128, 4], F32, tag="muT", bufs=1)
                isd = sm.tile([128, 4], F32, tag="isdT", bufs=1)
                nc.vector.tensor_scalar_mul(mu[:], ss[:, 0:4], 1.0 / N)
                nc.vector.tensor_scalar_mul(ss[:, 4:8], ss[:, 4:8], 1.0 / N)
                nc.vector.tensor_tensor(out=isd[:], in0=mu[:], in1=mu[:],
                                        op=AL.mult)
                nc.vector.tensor_tensor(out=isd[:], in0=ss[:, 4:8],
                                        in1=isd[:], op=AL.subtract)
                nc.vector.tensor_scalar_add(isd[:], isd[:], EPS_BN)
                nc.scalar.activation(isd[:], isd[:], ACTF.Ln)
                nc.vector.tensor_scalar_mul(isd[:], isd[:], -0.5)
                nc.scalar.activation(isd[:], isd[:], ACTF.Exp)
                scfT = sm.tile([128, 4], F32, tag="scfT", bufs=1)
                shfT = sm.tile([128, 4], F32, tag="shfT", bufs=1)
                nc.vector.tensor_tensor(out=scfT[:], in0=gT_sb[:],
                                        in1=isd[:], op=AL.mult)
                nc.vector.tensor_tensor(out=shfT[:], in0=scfT[:],
                                        in1=mu[:], op=AL.mult)
                nc.vector.tensor_tensor(out=shfT[:], in0=beT_sb[:],
                                        in1=shfT[:], op=AL.subtract)
                # scf/shf [128,4] -> per-col rows [1,128] -> bcast [128, 512]
                psc2 = psB.tile([1, 4, 128], F32, tag="sd", bufs=2,
                                name="psc2")
                psc3 = psB.tile([1, 4, 128], F32, tag="sd", bufs=2,
                                name="psc3")
                for k in range(4):
                    nc.tensor.transpose(psc2[:, k, :], scfT[:, k:k + 1],
                                        ident_sb[:])
                    nc.tensor.transpose(psc3[:, k, :], shfT[:, k:k + 1],
                                        ident_sb[:])
                row4 = sm.tile([1, 8, 128], F32, tag="row4", bufs=1)
                nc.vector.tensor_copy(row4[:, 0:4, :], psc2[:])
                nc.vector.tensor_copy(row4[:, 4:8, :], psc3[:])
                scT = sm.tile([128, FIN], F32, tag="scT", bufs=1)
                shT = sm.tile([128, FIN], F32, tag="shT", bufs=1)
                for k in range(4):
                    nc.gpsimd.partition_broadcast(
                        scT[:, 128 * k:128 * (k + 1)], row4[:, k, :])
                    nc.gpsimd.partition_broadcast(
                        shT[:, 128 * k:128 * (k + 1)], row4[:, 4 + k, :])
                for n in range(NT):
                    eng = nc.vector
                    eng1 = nc.gpsimd
                    v = sm.tile([128, FIN], BF16, tag="cht", name="v")
                    eng1.tensor_tensor(out=v[:, 0:fh],
                                       in0=ybuf[:, n, 0:fh],
                                       in1=scT[:, 0:fh], op=AL.mult)
                    eng1.tensor_tensor(out=v[:, 0:fh], in0=v[:, 0:fh],
                                       in1=shT[:, 0:fh], op=AL.add)
                    if ly == 3:
                        eng.tensor_copy(xnext[:, n, 0:fh], v[:, 0:fh])
                        continue
                    m = sm.tile([128, FIN], BF16, tag="che", name="m")
                    eng.tensor_scalar_min(m[:, 0:fh], v[:, 0:fh], 0.0)
                    nc.scalar.activation(m[:, 0:fh], m[:, 0:fh], ACTF.Exp)
                    xm = sm.tile([128, FIN], BF16, tag="chx", name="xm")
                    eng.tensor_tensor(out=xm[:, 0:fh], in0=m[:, 0:fh],
                                      in1=xprev[:, n, 0:fh], op=AL.add)
                    eng.tensor_scalar_max(v[:, 0:fh], v[:, 0:fh], 0.0)
                    eng.scalar_tensor_tensor(
                        out=xnext[:, n, 0:fh], in0=v[:, 0:fh], scalar=-1.0,
                        in1=xm[:, 0:fh], op0=AL.add, op1=AL.add)

            # ---------- transpose a -> aT (bf16) ----------
            def transpose_phase(abuf, aT):
                for n in range(NT):
                    for k in range(4):
                        tr = psB.tile([128, 128], BF16, tag="sd", bufs=2, name="tr")
                        nc.tensor.transpose(
                            tr[:], abuf[:, n, 128 * k:128 * (k + 1)],
                            identb_sb[:])
                        if k % 2:
                            nc.scalar.copy(aT[:, k, 128 * n:128 * (n + 1)], tr[:])
                        else:
                            nc.vector.tensor_copy(aT[:, k, 128 * n:128 * (n + 1)], tr[:])

            # =========== emit program ===========
            for _rep in range(repeat):
              xe, ybuf1, a1 = bufs[0], bufs[1], bufs[2]
              x0T_sb = xb.tile([128, 4, NT * 128], BF16, tag="lhsT",
                               name="x0T_sb")
              nc.sync.dma_start(x0T_sb[:],
                                x0T_d[:].rearrange("(k p) x -> p k x", p=128))
              sdloc = xb.tile([128, NT, 8], BF16, tag="sdloc", name="sdloc")
              wcat0 = cn.tile([128, 4, 1040], BF16, tag="wcat")
              nc.scalar.dma_start(wcat0[:], rhs0_d[:].rearrange("(k p) x -> p k x", p=128))

              # encoder + L1 h
              h_phase(1, x0T_sb, wcat0, FIN, SS12, eb1_sb, xe, sdloc)
              nc.gpsimd.collective_compute(
                  "AllGather", AL.bypass, replica_groups=RG,
                  ins=[cc_in[1][:]], outs=[cc_out[1][:]])
              pstA1 = psB.tile([128, 8], F32, tag="pstT", bufs=1, name="pstA1")
              edge_phase(1, ROW12, SS12, H, FIN, sdloc, ybuf1, pstA1, None)
              bn_chain(1, FIN, ybuf1, xe, a1, pstA1, None)

              # L2: a1 -> aT, h, edges (reuse xe buf as ybuf2, ybuf1 as a2)
              aT = xb.tile([128, 4, NT * 128], BF16, tag="lhsT")
              transpose_phase(a1, aT)
              wcat2 = cn.tile([128, 4, SS12], BF16, tag="wcat")
              nc.sync.dma_start(wcat2[:], w2_d[:].rearrange("(k p) x -> p k x", p=128))
              sdloc2 = xb.tile([128, NT, 8], BF16, tag="sdloc")
              h_phase(2, aT, wcat2, 0, SS12, None, None, sdloc2)
              nc.gpsimd.collective_compute(
                  "AllGather", AL.bypass, replica_groups=RG,
                  ins=[cc_in[2][:]], outs=[cc_out[2][:]])
              ybuf2, a2 = xe, ybuf1
              pstA2 = psB.tile([128, 8], F32, tag="pstT", bufs=1, name="pstA2")
              edge_phase(2, ROW12, SS12, H, FIN, sdloc2, ybuf2, pstA2, None)
              bn_chain(2, FIN, ybuf2, a1, a2, pstA2, None)

              # L3
              aT2 = xb.tile([128, 4, NT * 128], BF16, tag="lhsT")
              transpose_phase(a2, aT2)
              sdloc3 = xb.tile([128, NT, 1], BF16, tag="sdloc")
              h_phase(3, aT2, w3_sb, 0, SS3, None, None, sdloc3)
              nc.gpsimd.collective_compute(
                  "AllGather", AL.bypass, replica_groups=RG,
                  ins=[cc_in[3][:]], outs=[cc_out[3][:]])
              y3 = xb.tile([128, NT, C], F32, tag="y3")
              pstA3 = psA.tile([1, FIN], F32, tag="mm5", name="pstA3")
              pstB3 = psA.tile([1, FIN], F32, tag="mm5", name="pstB3")
              edge_phase(3, ROW3, SS3, 1, C, sdloc3, y3, pstA3, pstB3)

              # L3 stats + pooled sums, one AllGather for both
              stat3 = sm.tile([1, 2 * FIN], F32, tag="stat", name="stat3", bufs=1)
              nc.scalar.copy(stat3[:, 0:C], pstA3[:, 0:C])
              nc.scalar.copy(stat3[:, C:2 * C], pstB3[:, 0:C])
              # pooling on pre-BN y3: ygT[f, g] = sum_n y3[n, f] pool[n, g]
              pxg = psB.tile([C, G], F32, tag="sd", bufs=2, name="pxg")
              for n in range(NT):
                  nc.tensor.matmul(pxg[:], y3[:, n, :], pool_sb[:, n, :],
                                   start=(n == 0), stop=(n == NT - 1))
              xg = sm.tile([C, G], F32, tag="xg")
              nc.scalar.copy(xg[:], pxg[:])
              nc.sync.dma_start(ar3_in[0:C, :], xg[:])
              nc.sync.dma_start(ar3_in[C:C + 1, :], stat3[:, 0:C])
              nc.sync.dma_start(ar3_in[C + 1:C + 2, :], stat3[:, C:2 * C])
              nc.gpsimd.collective_compute(
                  "AllGather", AL.bypass, replica_groups=RG,
                  ins=[ar3_in[:]], outs=[ar3_out[:]])
              pooled8 = sm.tile([C, P, G], F32, tag="pooled8", bufs=1)
              nc.sync.dma_start(
                  pooled8[:, :, :],
                  ar3_out[:].rearrange("(r i) g -> i r g", r=P)[0:C])
              yg2 = sm.tile([C, G], F32, tag="xg2")
              nc.vector.tensor_reduce(
                  out=yg2[:, :],
                  in_=pooled8[:, :, :].rearrange("i r g -> i g r"),
                  axis=AX.X, op=AL.add)
              st8b = sm.tile([P, 2 * C], F32, tag="st8", bufs=1, name="st8b")
              nc.sync.dma_start(
                  st8b[:, :],
                  ar3_out[:].rearrange("(r i) g -> r (i g)", r=P)
                  [:, C * G:C * G + 2 * C])
              pm3 = psA.tile([1, FIN], F32, tag="mm5", name="pm3")
              nc.tensor.matmul(pm3[:, 0:2 * C], ones_cf[0:P, :],
                               st8b[:, :], start=True, stop=True)
              st3 = sm.tile([1, 2 * FIN], F32, tag="stat2", name="st3", bufs=1)
              nc.scalar.copy(st3[:, 0:2 * C], pm3[:, 0:2 * C])
              mu3 = st3[:, 0:C]
              ex23 = st3[:, C:2 * C]
              nc.vector.tensor_scalar_mul(mu3, mu3, 1.0 / N)
              nc.vector.tensor_scalar_mul(ex23, ex23, 1.0 / N)
              var3 = sm.tile([1, FIN], F32, tag="var", name="var3", bufs=1)
              nc.vector.tensor_tensor(out=var3[:, 0:C], in0=mu3, in1=mu3,
                                      op=AL.mult)
              nc.vector.tensor_tensor(out=var3[:, 0:C], in0=ex23,
                                      in1=var3[:, 0:C], op=AL.subtract)
              sd3 = sm.tile([1, FIN], F32, tag="sdv", name="sd3", bufs=1)
              nc.vector.tensor_scalar_add(var3[:, 0:C], var3[:, 0:C], EPS_BN)
              nc.scalar.activation(sd3[:, 0:C], var3[:, 0:C], ACTF.Ln)
              nc.vector.tensor_scalar_mul(sd3[:, 0:C], sd3[:, 0:C], -0.5)
              nc.scalar.activation(sd3[:, 0:C], sd3[:, 0:C], ACTF.Exp)
              g3_sb, be3_sb = bn_sb[3]
              scf3 = sm.tile([1, FIN], F32, tag="scf", name="scf3", bufs=1)
              nc.vector.tensor_tensor(out=scf3[:, 0:C], in0=g3_sb[:],
                                      in1=sd3[:, 0:C], op=AL.mult)
              shf3 = sm.tile([1, FIN], F32, tag="shf", name="shf3", bufs=1)
              nc.vector.tensor_tensor(out=shf3[:, 0:C], in0=scf3[:, 0:C],
                                      in1=mu3, op=AL.mult)
              nc.vector.tensor_tensor(out=shf3[:, 0:C], in0=be3_sb[:],
                                      in1=shf3[:, 0:C], op=AL.subtract)
              # transpose scf3/shf3 rows into per-partition columns [C, 1]
              psc = psB.tile([C, 1], F32, tag="Z", name="psc")
              nc.tensor.transpose(psc[:], scf3[:, 0:C], ident_sb[0:1, 0:1])
              scol = sm.tile([C, 1], F32, tag="scol", name="scol")
              nc.scalar.copy(scol[:], psc[:])
              psh = psB.tile([C, 1], F32, tag="Z", name="psh")
              nc.tensor.transpose(psh[:], shf3[:, 0:C], ident_sb[0:1, 0:1])
              shcol = sm.tile([C, 1], F32, tag="shcol", name="shcol")
              nc.scalar.copy(shcol[:], psh[:])
              # xgbn = yg2 * scol + shcol * indmat
              sh_t = sm.tile([C, G], F32, tag="shterm", name="sh_t")
              nc.vector.tensor_scalar_mul(sh_t[:], indmat_sb[:], shcol[:])
              xgbn = sm.tile([C, G], F32, tag="xgbn", name="xgbn")
              nc.vector.scalar_tensor_tensor(
                  out=xgbn[:], in0=yg2[:], scalar=scol[:], in1=sh_t[:],
                  op0=AL.mult, op1=AL.add)
              # outT[nc, g] = linW.T @ xgbn  (contract over f)
              pot = psB.tile([NCLS, G], F32, tag="sd", bufs=2, name="pot")
              nc.tensor.matmul(pot[:], linW_sb[:], xgbn[:], start=True,
                               stop=True)
              outT = sm.tile([NCLS, G], F32, tag="outT")
              nc.scalar.activation(outT[:], pot[:], ACTF.Identity,
                                   bias=linb_sb[:])
              pfin = psB.tile([G, NCLS], F32, tag="sd", bufs=2, name="pfin")
              nc.tensor.transpose(pfin[:], outT[:], ident_sb[0:NCLS, 0:NCLS])
              fin = sm.tile([G, NCLS], F32, tag="fin_sb")
              nc.vector.tensor_copy(fin[:], pfin[:])
              nc.sync.dma_start(out_d[:], fin[:])

        sched_state, snap = tc.schedule_and_allocate()
        nc._sched_state = sched_state
        nc._pred_ns = snap.time

    nc.finalize()
    return nc


_CACHE = {}


def _get_nc(T_key, TT, repeat=1):
    key = (T_key, repeat)
    if key not in _CACHE:
        _CACHE[key] = _build(list(T_key), TT, repeat)
    return _CACHE[key]


def make_in_maps(per_core, shared):
    return [dict(S=pc['S'], ST=pc['ST'], gidx=pc['gidx'],
                 x0T=pc['x0T'], pool=pc['pool'], **shared)
            for pc in per_core]


def kernel(**inputs):
    T_list, TT, per_core, shared = _prep(inputs)
    nc = _get_nc(tuple(T_list), TT)
    in_maps = make_in_maps(per_core, shared)
    res = run_bass_kernel_spmd(nc, in_maps, core_ids=list(range(P)))
    return np.asarray(res.results[0]['out'], np.float32)
